# revision 48
# baseline (speedup 1.0000x reference)
"""Causal multi-head attention block (B=4,S=2048,E=1024,H=16,D=64) on 8 trn2 cores.

Sharding: 4 batches x 2 head-groups (8 heads each) = 8 cores.
Each core: QKV projection for its (batch, head-group), causal attention,
partial output projection over its heads. Host sums the 2 partials per batch
(the "all-reduce after project_out" done at gather time) and adds b_out.

Layout: everything is computed transposed; no on-chip transposes anywhere.
  qkv^T[f, s] = W^T x^T   via matmul(lhsT=W[e,f], rhs=xT[e,s])
  V natural [s, f]        via matmul(lhsT=xT[e,s], rhs=Wv[e,f])
  scores^T[k, q] = K Q^T  via matmul(lhsT=KT[d,k], rhs=QT[d,q]) per head (d=64)
  softmax over k (= partition dim): exp on ACT (scale=1/sqrt(D) fused), the
  denominator comes free from a ones-column appended to V in the AV matmul,
  divide via DVE reciprocal + GpSimd partition_broadcast.
  ans^T[d, q]             via matmul(lhsT=[V|1][k, d+1], rhs=w^T[k, q])
  out^T[e, q] partial     via matmul(lhsT=Wout[f,e], rhs=ansT[f,q])

Projections run in compensated fp8 (e4m3) with DoubleRow perf mode: operands
are pre-scaled by powers of 2 (x*8, W*64) and split hi = fp8(a), lo =
fp8(a - hi); x@W ~ xh@Wh + xh@Wl + xl@Wh, three DoubleRow matmuls per
e-chunk-pair, each contracting 2x128 rows at 0.5 cycles/row -- 4x the bf16
FLOP rate, so the whole projection costs 0.75x its bf16 time at bf16-level
accuracy (residual quantization error ~0.1%). The 1/512 scale is folded into
the psum-drain tensor_scalar ops. SBUF tiles hold (hi_e0, hi_e1, lo_e0,
lo_e1) slot quads so all three matmuls address [p, 2, *] views of one tile.
The output projection does the same with ans split hi/lo at the softmax
divide (ans*8, W_out*64, 6 DoubleRow matmuls per out tile).

Scores for heads 0-3 run in plain fp8 DoubleRow (q/k pre-scaled by 8,
K8/Q8 stored as [32*h + d%32, dhalf, s] via a host-side Wq/Wk column
permutation; exp scale absorbs the 64x) -- 2x the bf16 rate at a measured
~1.4e-2 relative-error cost that fits the 2e-2 budget because each output
element mixes all 16 heads through the output projection (error scales as
sqrt(fp8-head fraction)). Heads 4-7 and all AV matmuls stay bf16: fp8
softmax weights/values cost 2-4e-2 (over budget) for only 2x.

All bf16 matmul operands run the PE at full rate even for narrow (<256)
outputs, so diagonal-band tiles use exact widths, and all DMA traffic
halves vs f32. Inputs are converted to bf16/fp8 on the host.

DMA strategy: every load is one batched transfer ([128, *] tiles built
with rearranges of the DRAM source), issued at kernel start across all
three issue paths (SP/ACT hwdge + Pool swdge); weights and all four x
blocks are SBUF-resident for the whole kernel. Block 0's x/wq/wk/wv are
split into 4 stripes each (one per e-chunk-pair, in separate tiles, so
dependency tracking is per-stripe) and block 0's projection runs 4 psum
groups wide with the pair-loop inner, consuming stripes as they land at
~the DMA supply rate. A short burst of dummy matmuls burns the PE p-state
ramp while the first stripes are in flight. Only output stores (batched in
pairs of e-tiles) remain inside the main loop.

Causality: k-tiles above the diagonal are skipped; diagonal-band tiles use
exact-width matmuls/exp (columns >= j*128) plus a [128,128] triangle mask.

The head-pair loop is software-pipelined three tiles deep, and across
q-block boundaries five tiles deep: the next pair's (or next block's pair
0's) first score/exp tiles are emitted before the current pair's AV drain
and epilogue, so the ACT engine (whose exp backlog gates the final divide
chain) never starves at pair or block boundaries.

Scheduling: the attention inner loop is ACT(exp)-limited while projections
are pure PE work, so projection/output-projection generators are interleaved
(paced round-robin) into each attention block's instruction stream to keep
the in-order PE engine saturated. The final block's output projection is
split so its tail executes during the last softmax epilogue's divide chain.
"""

import numpy as np

B, S, E, H, D = 4, 2048, 1024, 16, 64
NCORES = 8
HG = 2                 # head groups (tensor parallel)
HC = H // HG           # 8 heads per core
FQ = HC * D            # 512 local features per q/k/v
P, NB = 128, 512       # partition tile, free-dim block
ET, ST, KTN, FT = E // P, S // NB, S // P, FQ // P   # 8, 4, 16, 4
EP = ET // 2           # e-chunk pairs (4)
SX, SW = 8.0, 64.0     # fp8 pre-scales for x / weights
INV = 1.0 / (SX * SW)  # 1/512 drain scale

_cache = {}
FACS = {0: 1.20, 1: 1.20, 2: 1.00, 3: 1.05}  # filler pacing per q-block
NSC = 4                # heads with fp8 DoubleRow score matmuls (0 or 4)
SQ = 8.0               # q/k fp8 pre-scale for fp8-score heads
XC1, XC2 = 8, 14       # cross-block seed caps (pend-drain / epilogue)
HOIST = 3              # next-pair hoist depth


def _build():
    from contextlib import ExitStack
    import concourse.tile as tile
    import concourse.mybir as mybir
    from concourse import bacc

    dt = mybir.dt
    f32, bf16, f8 = dt.float32, dt.bfloat16, dt.float8e4
    AF = mybir.ActivationFunctionType
    ALU = mybir.AluOpType
    DR = mybir.MatmulPerfMode.DoubleRow
    SCALE = 0.125  # 1/sqrt(D)

    nc = bacc.Bacc("TRN2", target_bir_lowering=False, debug=False,
                   num_devices=NCORES)

    # host-packed fp8 hi/lo slot layouts (contiguous DMA images):
    # xsd[s] = block-0 stripe s [P, 4, NB]; xad[i] = block 1+i [P, 16*NB]
    xsd = nc.dram_tensor("xsd", [EP, P, 4 * NB], f8,
                         kind="ExternalInput").ap()
    xad = nc.dram_tensor("xad", [ST - 1, P, 16 * NB], f8,
                         kind="ExternalInput").ap()
    wqd = nc.dram_tensor("wqd", [EP, P, 4 * FQ], f8,
                         kind="ExternalInput").ap()
    wkd = nc.dram_tensor("wkd", [EP, P, 4 * FQ], f8,
                         kind="ExternalInput").ap()
    wvd = nc.dram_tensor("wvd", [EP, P, 4 * FQ], f8,
                         kind="ExternalInput").ap()
    wod = nc.dram_tensor("wod", [2, P, 4 * E], f8,
                         kind="ExternalInput").ap()
    msk = nc.dram_tensor("msk", [P, P], bf16, kind="ExternalInput").ap()
    bq = nc.dram_tensor("bq", [FQ], f32, kind="ExternalInput").ap()
    bk = nc.dram_tensor("bk", [FQ], f32, kind="ExternalInput").ap()
    bvb = nc.dram_tensor("bvb", [P, FQ], f32, kind="ExternalInput").ap()
    outT = nc.dram_tensor("outT", [E, S], bf16, kind="ExternalOutput").ap()

    with tile.TileContext(nc) as tc:
        with ExitStack() as ctx:
            pers = ctx.enter_context(tc.tile_pool(name="pers", bufs=1))
            pqts = ctx.enter_context(tc.tile_pool(name="pqts", bufs=2))
            pwe = ctx.enter_context(tc.tile_pool(name="pwe", bufs=16))
            pans = ctx.enter_context(tc.tile_pool(name="pans", bufs=4))
            pepi = ctx.enter_context(tc.tile_pool(name="pepi", bufs=4))
            pout = ctx.enter_context(tc.tile_pool(name="pout", bufs=4))
            ps1 = ctx.enter_context(
                tc.tile_pool(name="ps1", bufs=2, space="PSUM"))
            sps = ctx.enter_context(
                tc.tile_pool(name="sps", bufs=2, space="PSUM"))
            avps = ctx.enter_context(
                tc.tile_pool(name="avps", bufs=2, space="PSUM"))

            # ---- resident tensors -------------------------------------
            # heads 0-3 keep K in the fp8 DoubleRow layout K8[p, dhalf, k]
            # with p = 32*h + d%32 (host reorders Wq/Wk columns to match);
            # heads 4-7 keep the bf16 [d, k] layout.
            K8 = pers.tile([P, 2 * S], f8, tag="k8", name="k8")
            KT = [None, None] + [
                pers.tile([P, S], bf16, tag=f"kt{i}", name=f"kt{i}")
                for i in range(2, FT)]
            Vp = [pers.tile([P, HC * (D + 1)], bf16, tag=f"vp{i}",
                            name=f"vp{i}") for i in range(KTN)]
            # x blocks 1..3: [P, pair(4) x slot(4) x NB] fp8, slot order
            # (hi_e0, hi_e1, lo_e0, lo_e1) per chunk pair
            XA = [None] + [pers.tile([P, 16 * NB], f8, tag=f"xa{i}",
                                     name=f"xa{i}") for i in range(1, ST)]
            # block-0 stripes: stripe s = chunk pair (2s, 2s+1)
            XS = [pers.tile([P, 4 * NB], f8, tag=f"xs{i}", name=f"xs{i}")
                  for i in range(EP)]
            WQS = [pers.tile([P, 4 * FQ], f8, tag=f"wqs{i}", name=f"wqs{i}")
                   for i in range(EP)]
            WKS = [pers.tile([P, 4 * FQ], f8, tag=f"wks{i}", name=f"wks{i}")
                   for i in range(EP)]
            WVS = [pers.tile([P, 4 * FQ], f8, tag=f"wvs{i}", name=f"wvs{i}")
                   for i in range(EP)]
            # out-projection weights: j = f-chunk pair (2j, 2j+1)
            WOA = [pers.tile([P, 4 * E], f8, tag=f"woa{j}", name=f"woa{j}")
                   for j in range(2)]

            def _xv(sb, a):
                """[p, 4, NB] slot view of chunk pair a in s-block sb."""
                if sb == 0:
                    return XS[a][:].rearrange("p (t s) -> p t s", s=NB)
                v = XA[sb][:].rearrange("p (a t s) -> p a t s", t=4, s=NB)
                return v[:, a]

            def _wv(W, a):
                """[p, 4, FQ] slot view of weight chunk pair a."""
                return W[a][:].rearrange("p (t f) -> p t f", f=FQ)

            bqt = pers.tile([P, FT], f32, tag="bqt")
            bkt = pers.tile([P, FT], f32, tag="bkt")
            bvt = pers.tile([P, FQ], f32, tag="bvt")
            onesf = pers.tile([P, HC], bf16, tag="onesf")
            mtri = pers.tile([P, P], bf16, tag="mtri")
            dum = pers.tile([P, NB], bf16, tag="dum")
            obt = [pers.tile([P, NB], bf16, tag=f"obt{i}", name=f"obt{i}")
                   for i in range(2)]
            nc.vector.memset(dum[:], 1.0)
            nc.vector.memset(onesf[:], 1.0)

            # ---- startup DMA plan -------------------------------------
            # 4 stripes each for block-0 x / wq / wk / wv (so the first
            # projection matmuls start supply-paced ~3us in), one batched
            # transfer for everything else. Queues: SP=x,
            # ACT=wq+biases+mask, Pool-SWDGE=wk+wv+wo.
            for s in range(EP):
                nc.sync.dma_start(XS[s][:], xsd[s])
                nc.scalar.dma_start(WQS[s][:], wqd[s])
            # small tiles go through SWDGE first so their transfers slot in
            # between the early x/wq stripes without head-of-line blocking
            nc.gpsimd.dma_start(bqt[:], bq.rearrange("(a p) -> p a", p=P))
            nc.gpsimd.dma_start(bkt[:], bk.rearrange("(a p) -> p a", p=P))
            for s in range(EP):
                nc.gpsimd.dma_start(WKS[s][:], wkd[s])
                (nc.scalar if s % 2 else nc.sync).dma_start(
                    WVS[s][:], wvd[s])
            nc.gpsimd.dma_start(mtri[:], msk[:])
            nc.gpsimd.dma_start(bvt[:], bvb[:])
            for sb in range(1, ST):
                nc.sync.dma_start(XA[sb][:], xad[sb - 1])
            for j in range(2):
                nc.gpsimd.dma_start(WOA[j][:], wod[j])

            # per-block state shared between generators
            QTS = {}    # sb -> [q8, q8, qt2, qt3] (hp-indexed)
            ATS = {}    # qb -> [2 tiles: [P, 4, NB] (hi2j, hi2j+1, lo..)]
            XSEED = []  # cross-block hoisted score tiles (next qb, pair 0)

            def _drain_q(sb, ft, ps):
                if ft < 2:
                    if ft == 0:
                        q8 = pqts.tile([P, 2 * NB], f8, tag="q8",
                                       name=f"q8_{sb}")
                        QTS.setdefault(sb, []).append(q8)
                    else:
                        q8 = QTS[sb][0]
                        QTS[sb].append(q8)
                    v = q8[:].rearrange("p (t s) -> p t s", s=NB)
                    nc.vector.tensor_scalar(
                        v[:, ft], ps[:], INV * SQ, bqt[:, ft:ft + 1],
                        op0=ALU.mult, op1=ALU.add)
                else:
                    qt = pqts.tile([P, NB], bf16, tag=f"qts{ft}",
                                   name=f"qts{ft}_{sb}")
                    nc.vector.tensor_scalar(
                        qt[:], ps[:], INV, bqt[:, ft:ft + 1],
                        op0=ALU.mult, op1=ALU.add)
                    QTS.setdefault(sb, []).append(qt)

            def _drain_k(sb, ft, ps):
                cols = slice(sb * NB, (sb + 1) * NB)
                if ft < 2:
                    v = K8[:].rearrange("p (t k) -> p t k", k=S)
                    nc.vector.tensor_scalar(
                        v[:, ft, cols], ps[:], INV * SQ,
                        bkt[:, ft:ft + 1], op0=ALU.mult, op1=ALU.add)
                else:
                    nc.vector.tensor_scalar(
                        KT[ft][:, cols], ps[:], INV, bkt[:, ft:ft + 1],
                        op0=ALU.mult, op1=ALU.add)

            def emit3(ps, wvw, xvw, cols, first, last):
                """3 comp DoubleRow matmuls for one chunk pair into ps."""
                nc.tensor.matmul(ps, wvw[:, 0:2, cols[0]:cols[1]],
                                 xvw[:, 0:2, :], start=first, stop=False,
                                 perf_mode=DR)
                nc.tensor.matmul(ps, wvw[:, 2:4, cols[0]:cols[1]],
                                 xvw[:, 0:2, :], start=False, stop=False,
                                 perf_mode=DR)
                nc.tensor.matmul(ps, wvw[:, 0:2, cols[0]:cols[1]],
                                 xvw[:, 2:4, :], start=False, stop=last,
                                 perf_mode=DR)

            def emit3v(ps, xvw, wvw, stl, first, last):
                """3 comp DoubleRow matmuls, V orientation (x stationary)."""
                c0, c1 = stl * P, (stl + 1) * P
                nc.tensor.matmul(ps, xvw[:, 0:2, c0:c1], wvw[:, 0:2, :],
                                 start=first, stop=False, perf_mode=DR)
                nc.tensor.matmul(ps, xvw[:, 0:2, c0:c1], wvw[:, 2:4, :],
                                 start=False, stop=False, perf_mode=DR)
                nc.tensor.matmul(ps, xvw[:, 2:4, c0:c1], wvw[:, 0:2, :],
                                 start=False, stop=last, perf_mode=DR)

            def proj0():
                """QKV projection of s-block 0, emitted standalone before
                the main loop. Runs 4 psum groups wide (ps1 + borrowed
                score-psum banks, idle until attention starts) so every
                arriving x/w DMA stripe is consumed with 12 matmuls
                (~1.28us) -- close to the ~1.46us/stripe supply rate, so
                the PE tracks the DMA stream with no re-read passes."""
                POOL6 = [(ps1, "ps"), (ps1, "ps"), (sps, "sp"),
                         (sps, "sp"), (avps, "av"), (avps, "av")]
                qoff = [0]

                def quad():
                    off = qoff[0]
                    qoff[0] = (off + 4) % 6
                    return [POOL6[(off + k) % 6][0].tile(
                                [P, NB], f32, tag=POOL6[(off + k) % 6][1],
                                name=f"p0_{off}_{k}")
                            for k in range(4)]
                for wts, dst in ((WQS, "q"), (WKS, "k")):
                    ps = quad()
                    for a in range(EP):
                        wvw = _wv(wts, a)
                        xvw = _xv(0, a)
                        for ft in range(FT):
                            emit3(ps[ft][:], wvw, xvw,
                                  (ft * P, (ft + 1) * P),
                                  first=(a == 0), last=(a == EP - 1))
                    for ft in range(FT):
                        if dst == "q":
                            _drain_q(0, ft, ps[ft])
                        else:
                            _drain_k(0, ft, ps[ft])
                ps = quad()
                for a in range(EP):
                    wvw = _wv(WVS, a)
                    xvw = _xv(0, a)
                    for stl in range(ST):
                        emit3v(ps[stl][:], xvw, wvw, stl,
                               first=(a == 0), last=(a == EP - 1))
                for stl in range(ST):
                    _vp_write(stl, ps[stl])

            def _vp_write(st, ps):
                vview = Vp[st][:].rearrange("p (h c) -> p h c", c=D + 1)
                nc.vector.tensor_copy(
                    vview[:, :, D:D + 1],
                    onesf[:].rearrange("p (h c) -> p h c", c=1))
                nc.vector.scalar_tensor_tensor(
                    vview[:, :, 0:D], ps[:], INV,
                    bvt[:].rearrange("p (h d) -> p h d", d=D),
                    op0=ALU.mult, op1=ALU.add)

            def proj_gen(sb):
                """QKV projection of s-block sb>=1 (all inputs resident).
                Yields between PE chunks; single open psum at a time so the
                shared ps1 ring stays safe under filler interleaving."""
                for ft in range(FT):
                    ps = ps1.tile([P, NB], f32, tag="ps", name=f"psq{ft}_{sb}")
                    for a in range(EP):
                        emit3(ps[:], _wv(WQS, a), _xv(sb, a),
                              (ft * P, (ft + 1) * P),
                              first=(a == 0), last=(a == EP - 1))
                        if a == 1:
                            yield
                    _drain_q(sb, ft, ps)
                    yield
                for ft in range(FT):
                    ps = ps1.tile([P, NB], f32, tag="ps", name=f"psk{ft}_{sb}")
                    for a in range(EP):
                        emit3(ps[:], _wv(WKS, a), _xv(sb, a),
                              (ft * P, (ft + 1) * P),
                              first=(a == 0), last=(a == EP - 1))
                        if a == 1:
                            yield
                    _drain_k(sb, ft, ps)
                    yield
                for stl in range(ST):
                    ps = ps1.tile([P, NB], f32, tag="ps",
                                  name=f"psv{stl}_{sb}")
                    for a in range(EP):
                        emit3v(ps[:], _xv(sb, a), _wv(WVS, a), stl,
                               first=(a == 0), last=(a == EP - 1))
                        if a == 1:
                            yield
                    _vp_write(ST * sb + stl, ps)
                    yield

            def attn_gen(qb):
                """Attention for q-block qb. Yields once per kt step.

                The head-pair loop is software-pipelined: the NEXT pair's
                first score/exp tile is emitted before this pair's AV drain
                and epilogue, so the ACT engine never starves at pair
                boundaries (its backlog gates the final divide chain)."""
                nkt = ST * (qb + 1)
                ATS[qb] = [pans.tile([P, 4 * NB], f8, tag=f"at2{j}",
                                     name=f"at2{j}_{qb}") for j in range(2)]

                def tile_step(hp, kt, qb2=qb):
                    QT2 = QTS[qb2]
                    j = kt - ST * qb2
                    c0 = j * P if j >= 0 else 0
                    # both heads of the pair share one 2-bank psum tile
                    # and a single strided exp call
                    sp = sps.tile([P, 2 * NB], f32, tag="sp",
                                  name=f"sp{qb2}_{hp}_{kt}")
                    if hp < 2:
                        # fp8 DoubleRow scores: one matmul per head,
                        # contracting both d-halves at 0.5 cycles/row
                        k8v = K8[:].rearrange("p (t k) -> p t k", k=S)
                        q8v = QT2[0][:].rearrange("p (t s) -> p t s", s=NB)
                        for i in range(2):
                            h = 2 * hp + i
                            hs = slice(h * 32, (h + 1) * 32)
                            nc.tensor.matmul(
                                sp[:, i * NB + c0:(i + 1) * NB],
                                k8v[hs, :, kt * P:(kt + 1) * P],
                                q8v[hs, :, c0:NB],
                                start=True, stop=True, perf_mode=DR,
                                tile_position=(h * 32, 0))
                    else:
                        for i in range(2):
                            nc.tensor.matmul(
                                sp[:, i * NB + c0:(i + 1) * NB],
                                KT[hp][i * D:(i + 1) * D,
                                       kt * P:(kt + 1) * P],
                                QT2[hp][i * D:(i + 1) * D, c0:NB],
                                start=True, stop=True)
                    w = pwe.tile([P, 2 * NB], bf16, tag="w",
                                 name=f"w{qb2}_{hp}_{kt}")
                    spv = sp[:].rearrange("p (h q) -> p h q", h=2)
                    wv_ = w[:].rearrange("p (h q) -> p h q", h=2)
                    nc.scalar.activation(wv_[:, :, c0:NB],
                                         spv[:, :, c0:NB],
                                         AF.Exp,
                                         scale=(SCALE / (SQ * SQ)
                                                if hp < 2 else SCALE))
                    if j >= 0:
                        nc.vector.tensor_mul(
                            wv_[:, :, c0:c0 + P], wv_[:, :, c0:c0 + P],
                            mtri[:]
                            .rearrange("p (a q) -> p a q", a=1)
                            .broadcast_to([P, 2, P]))
                    return (kt, c0, w)

                hoist = list(XSEED)
                del XSEED[:]
                for hp in range(FT):
                    av = [avps.tile([D + 1, NB], f32, tag="av",
                                    name=f"av{qb}_{hp}_{i}")
                          for i in range(2)]

                    def emit_av(ent, last, av=av, hp=hp):
                        k0, pc0, w0 = ent
                        for i in range(2):
                            nc.tensor.matmul(
                                av[i][:, pc0:NB],
                                Vp[k0][:, (2 * hp + i) * (D + 1):
                                                (2 * hp + i + 1) * (D + 1)],
                                w0[:, i * NB + pc0:(i + 1) * NB],
                                start=(k0 == 0), stop=last)

                    pend = list(hoist)
                    ktlo = len(hoist)
                    hoist = []
                    for kt in range(ktlo, nkt):
                        pend.append(tile_step(hp, kt))
                        if len(pend) > 2:
                            emit_av(pend.pop(0), last=False)
                        if kt == nkt - 1:
                            if hp + 1 < FT:
                                hoist.append(tile_step(hp + 1, 0))
                            elif (qb + 1 < ST
                                  and len(QTS.get(qb + 1, [])) == FT):
                                XSEED.append(tile_step(0, 0, qb + 1))
                        yield
                    while pend:
                        ent = pend.pop(0)
                        emit_av(ent, last=not pend)
                        if 0 < len(hoist) < min(HOIST, nkt) \
                                and hp + 1 < FT:
                            hoist.append(tile_step(hp + 1, len(hoist)))
                        elif (hp + 1 == FT and qb + 1 < ST
                              and len(QTS.get(qb + 1, [])) == FT):
                            nx = min(XC1, ST * (qb + 2))
                            for _x in range(2):
                                if 0 < len(XSEED) < nx:
                                    XSEED.append(
                                        tile_step(0, len(XSEED), qb + 1))
                        yield
                    # epilogue per head: tmp = copy(av) (frees the psum
                    # slot fast, no divide-chain wait), then in-place
                    # tmp = (tmp*8)/Z; at_hi = fp8(tmp), at_lo =
                    # fp8(tmp - at_hi) rounded pair-wide below.
                    tmpb = pepi.tile([P, NB], bf16, tag="tmpb",
                                     name=f"tmpb{qb}_{hp}")
                    # the very last pair's chain is the exposed tail: route
                    # its copies through ACT (idle after the final exp) and
                    # round hi/lo per head so head 0's rounding overlaps
                    # head 1's divide chain.
                    last_pair = (qb == ST - 1 and hp == FT - 1)
                    on_act = qb <= 1 or last_pair
                    j, jj = hp // 2, hp % 2
                    atv = ATS[qb][j][:].rearrange("p (t s) -> p t s", s=NB)
                    for i in range(2):
                        se = pepi.tile([1, NB], f32, tag="se",
                                       name=f"se{qb}_{hp}_{i}")
                        if on_act:
                            nc.scalar.copy(se[:], av[i][D:D + 1, :])
                            nc.scalar.copy(tmpb[i * D:(i + 1) * D, :],
                                           av[i][0:D, :])
                        else:
                            nc.vector.tensor_copy(se[:], av[i][D:D + 1, :])
                            nc.vector.tensor_copy(
                                tmpb[i * D:(i + 1) * D, :], av[i][0:D, :])
                        nc.vector.reciprocal_approx_fast(se[:], se[:])
                        bch = pepi.tile([P, NB], f32, tag="bch",
                                        name=f"bch{qb}_{hp}_{i}")
                        nc.gpsimd.partition_broadcast(
                            bch[0:(i + 1) * D, :], se[:],
                            channels=(i + 1) * D)
                        nc.vector.scalar_tensor_tensor(
                            tmpb[i * D:(i + 1) * D, :],
                            tmpb[i * D:(i + 1) * D, :], SX,
                            bch[i * D:(i + 1) * D, :],
                            op0=ALU.mult, op1=ALU.mult)
                        if last_pair:
                            hs = slice(i * D, (i + 1) * D)
                            nc.vector.tensor_copy(atv[hs, jj], tmpb[hs, :])
                            nc.vector.tensor_tensor(
                                atv[hs, 2 + jj], tmpb[hs, :], atv[hs, jj],
                                op=ALU.subtract)
                        if (hp + 2 >= FT and qb + 1 < ST
                                and len(QTS.get(qb + 1, [])) == FT):
                            nx = min(XC2, ST * (qb + 2))
                            for _x in range(2):
                                if len(XSEED) < nx \
                                        and (XSEED or hp + 1 == FT):
                                    XSEED.append(
                                        tile_step(0, len(XSEED), qb + 1))
                        yield
                    if not last_pair:
                        # pair-wide hi/lo rounding (both heads at once)
                        nc.vector.tensor_copy(atv[:, jj], tmpb[:])
                        nc.vector.tensor_tensor(
                            atv[:, 2 + jj], tmpb[:], atv[:, jj],
                            op=ALU.subtract)
                    yield

            def store_pair(qb, et, ob, eng=None):
                # all loads are issued up-front, so SP.SEQ is free during
                # the main loop; SWDGE stores would block Pool.SEQ (and the
                # softmax broadcasts) while waiting for staging data
                (eng or nc.sync).dma_start(
                    outT[(et - 1) * P:(et + 1) * P,
                         qb * NB:(qb + 1) * NB]
                    .rearrange("(a p) s -> p a s", p=P),
                    ob[:].rearrange("p (a s) -> p a s", s=NB))

            def out_mm(po, qb, et, jset=(0, 1), first=True, last=True):
                """comp DoubleRow out-projection matmuls for e-tile et."""
                for jx, j in enumerate(jset):
                    wvw = WOA[j][:].rearrange("p (t e) -> p t e", e=E)
                    atv = ATS[qb][j][:].rearrange("p (t s) -> p t s", s=NB)
                    c = (et * P, (et + 1) * P)
                    nc.tensor.matmul(
                        po, wvw[:, 0:2, c[0]:c[1]], atv[:, 0:2, :],
                        start=(first and jx == 0), stop=False, perf_mode=DR)
                    nc.tensor.matmul(
                        po, wvw[:, 2:4, c[0]:c[1]], atv[:, 0:2, :],
                        start=False, stop=False, perf_mode=DR)
                    nc.tensor.matmul(
                        po, wvw[:, 0:2, c[0]:c[1]], atv[:, 2:4, :],
                        start=False,
                        stop=(last and jx == len(jset) - 1), perf_mode=DR)

            def out_gen(qb, ets=None, act_copy=False):
                """Output projection of q-block qb. Yields per e-tile.
                Stores are batched in pairs of e-tiles. act_copy routes the
                psum drains through ACT (for tail portions emitted after the
                last exp, when ACT is idle but DVE is still busy)."""
                ob = None
                for et in (range(ET) if ets is None else ets):
                    if et % 2 == 0:
                        ob = pout.tile([P, 2 * NB], bf16, tag="ob",
                                       name=f"ob{qb}_{et}")
                    po = ps1.tile([P, NB], f32, tag="ps",
                                  name=f"po{qb}_{et}")
                    out_mm(po[:], qb, et)
                    if act_copy:
                        nc.scalar.mul(
                            ob[:, (et % 2) * NB:(et % 2 + 1) * NB], po[:],
                            INV)
                    else:
                        nc.vector.tensor_scalar(
                            ob[:, (et % 2) * NB:(et % 2 + 1) * NB], po[:],
                            INV, None, op0=ALU.mult)
                    if et % 2 == 1:
                        store_pair(qb, et, ob)
                    yield

            O3 = {}

            def out3_a():
                """Final-block e-tiles 0-5 open with the j=0 (head pairs
                0-1) halves: pure PE work depending only on those pairs.
                Emitted right after the last pair's AV drain so it executes
                during that pair's divide chain. The open groups borrow
                attention's score psum slots."""
                for et in (0, 1, 2, 3, 4, 5):
                    pool, tg = ((ps1, "ps") if et < 2 else
                                (sps, "sp") if et < 4 else (avps, "av"))
                    po = pool.tile([P, NB], f32, tag=tg, name=f"po3a_{et}")
                    O3[et] = po[:]
                    out_mm(po[:], ST - 1, et, jset=(0,), first=True,
                           last=False)

            def out3():
                """Final block: j=1 closers for e-tiles 0-5, full
                accumulations for e-tiles 6-7, stores batched in pairs with
                single-tile tail stores on alternating queues."""
                qb = ST - 1
                out3_a()
                pos = O3
                ob = None
                for et in range(ET):
                    if et < 6:
                        po = pos[et]
                        out_mm(po, qb, et, jset=(1,), first=False,
                               last=True)
                    else:
                        pool, tg = (ps1, "ps") if et == 6 else (sps, "sp")
                        po = pool.tile([P, NB], f32, tag=tg,
                                       name=f"po3b_{et}")[:]
                        out_mm(po, qb, et)
                    if et < 6:
                        if et % 2 == 0:
                            ob = pout.tile([P, 2 * NB], bf16, tag="ob",
                                           name=f"ob{qb}_{et}")
                            nc.scalar.mul(ob[:, 0:NB], po, INV)
                        else:
                            nc.vector.tensor_scalar(
                                ob[:, NB:2 * NB], po, INV, None,
                                op0=ALU.mult)
                            store_pair(qb, et, ob)
                    else:
                        # drain tail: single-tile stores on alternating
                        # queues so the last transfers issue immediately
                        ob = obt[et - 6]
                        if et == 6:
                            nc.scalar.mul(ob[:], po, INV)
                        else:
                            nc.vector.tensor_scalar(
                                ob[:], po, INV, None, op0=ALU.mult)
                        (nc.gpsimd if et == 6 else nc.sync).dma_start(
                            outT[et * P:(et + 1) * P,
                                 qb * NB:(qb + 1) * NB], ob[:])

            def drain(g):
                for _ in g:
                    pass

            # warmup: burn the PE p-state ramp while the first input
            # stripes are still in flight, so real matmuls start full-rate
            for i in range(4):
                dp = avps.tile([8, NB], f32, tag="av", name=f"dummy{i}")
                nc.tensor.matmul(dp[:], dum[:, 0:8], dum[:],
                                 start=True, stop=True)
            proj0()
            # Filler plan: spread PE-only work over each attention block to
            # absorb the ACT(exp) deficit; OUT(1)/OUT(2) go to attention(3),
            # which has no projection work left to hide exp latency.
            plans = {
                0: ([lambda: proj_gen(1)], 24),
                1: ([lambda: proj_gen(2)], 24),
                2: ([lambda: proj_gen(3)], 24),
                3: ([lambda: out_gen(0), lambda: out_gen(1),
                     lambda: out_gen(2)], 24),
            }
            for qb in range(ST):
                mk, nf = plans[qb]
                fillers = [m() for m in mk]
                na = 4 * (ST * (qb + 1) + 6)
                fac = FACS[qb]
                rate = fac * nf / na
                acc, fi = 0.0, 0
                for _ in attn_gen(qb):
                    acc += rate
                    while acc >= 1.0 and fillers:
                        acc -= 1.0
                        f = fillers[fi % len(fillers)]
                        fi += 1
                        try:
                            next(f)
                        except StopIteration:
                            fillers.remove(f)
                for f in fillers:
                    drain(f)
            out3()
    nc.compile()
    return nc


def _mask_tri():
    import ml_dtypes
    kp = np.arange(P)[:, None]
    qf = np.arange(P)[None, :]
    return (qf >= kp).astype(ml_dtypes.bfloat16)


def _qk_perm():
    """Column permutation for Wq/Wk: features of heads 0-3 reordered to
    (d-half, 32*h + d%32) so projection psums land in the fp8 DoubleRow
    score layout; heads 4-7 unchanged."""
    perm = np.arange(FQ)
    for newcol in range(2 * P):
        ft, p = divmod(newcol, P)
        h, dd = divmod(p, 32)
        perm[newcol] = h * D + ft * 32 + dd
    return perm


def _qk_bias(bvec, perm):
    """bias vector reordered like the W columns, with the fp8-score heads'
    entries pre-scaled by SQ (their drains fold q8 = SQ*(q + bias))."""
    b = np.ascontiguousarray(bvec.reshape(FQ)[perm])
    b[:2 * P] *= SQ
    return b


def _hilo(a):
    """fp8 hi/lo split: a ~ hi + lo, both e4m3."""
    import ml_dtypes
    f8 = ml_dtypes.float8_e4m3fn
    a = np.ascontiguousarray(a, dtype=np.float32)
    hi = a.astype(f8)
    lo = (a - hi.astype(np.float32)).astype(f8)
    return hi, lo


def _slotpack(hi, lo, cols):
    """[rows=2*P, cols] hi/lo planes -> [P, 4, cols] slot quad
    (hi_e0, hi_e1, lo_e0, lo_e1)."""
    out = np.empty((P, 4, cols), dtype=hi.dtype)
    out[:, 0] = hi[0:P]
    out[:, 1] = hi[P:2 * P]
    out[:, 2] = lo[0:P]
    out[:, 3] = lo[P:2 * P]
    return out


def _pack_w(w):
    """[E or FQ, cols] scaled weight -> [npairs, P, 4*cols] stripe images."""
    hi, lo = _hilo(w)
    n = w.shape[0] // (2 * P)
    return np.stack([
        _slotpack(hi[2 * s * P:(2 * s + 2) * P],
                  lo[2 * s * P:(2 * s + 2) * P],
                  w.shape[1]).reshape(P, -1)
        for s in range(n)])


def _pack_x(xT):
    """[E, S] scaled x^T -> (stripe images [EP, P, 4*NB],
    block images [ST-1, P, 16*NB])."""
    hi, lo = _hilo(xT)
    xs = np.stack([
        _slotpack(hi[2 * s * P:(2 * s + 2) * P, 0:NB],
                  lo[2 * s * P:(2 * s + 2) * P, 0:NB], NB).reshape(P, -1)
        for s in range(EP)])
    xa = np.empty((ST - 1, P, 16 * NB), dtype=hi.dtype)
    for sb in range(1, ST):
        c = slice(sb * NB, (sb + 1) * NB)
        blk = np.stack([
            _slotpack(hi[2 * a * P:(2 * a + 2) * P, c],
                      lo[2 * a * P:(2 * a + 2) * P, c], NB)
            for a in range(EP)], axis=1)          # [P, 4, 4, NB]
        xa[sb - 1] = blk.reshape(P, -1)
    return xs, xa


def kernel(x, W_qkv, b_qkv, W_out, b_out):
    from concourse.bass_utils import run_bass_kernel_spmd

    if "nc" not in _cache:
        _cache["nc"] = _build()
    nc = _cache["nc"]

    x = np.asarray(x, dtype=np.float32)
    W_qkv = np.asarray(W_qkv, dtype=np.float32)
    b_qkv = np.asarray(b_qkv, dtype=np.float32)
    W_out = np.asarray(W_out, dtype=np.float32)
    b_out = np.asarray(b_out, dtype=np.float32)

    mtri = _mask_tri()
    perm = _qk_perm()
    in_maps = []
    for c in range(NCORES):
        b, g = c % B, c // B
        hs = slice(g * HC, (g + 1) * HC)
        Wl = W_qkv[:, :, hs, :]                       # [E, 3, HC, D]
        xs_im, xa_im = _pack_x(x[b].T * SX)
        in_maps.append({
            "xsd": xs_im,
            "xad": xa_im,
            "wqd": _pack_w(Wl[:, 0].reshape(E, FQ)[:, perm] * SW),
            "wkd": _pack_w(Wl[:, 1].reshape(E, FQ)[:, perm] * SW),
            "wvd": _pack_w(Wl[:, 2].reshape(E, FQ) * SW),
            "wod": _pack_w(W_out[hs].reshape(FQ, E) * SW),
            "msk": mtri,
            "bq": _qk_bias(b_qkv[0, hs], perm),
            "bk": _qk_bias(b_qkv[1, hs], perm),
            "bvb": np.broadcast_to(b_qkv[2, hs].reshape(1, FQ),
                                   (P, FQ)).copy(),
        })

    try:
        res = run_bass_kernel_spmd(nc, in_maps, core_ids=list(range(NCORES)))
    except Exception:
        # transient device wedges (NRT_EXEC_UNIT_UNRECOVERABLE) clear on retry
        res = run_bass_kernel_spmd(nc, in_maps, core_ids=list(range(NCORES)))
    _cache["last_results"] = res
    out = np.empty((B, S, E), dtype=np.float32)
    for b in range(B):
        out[b] = (res.results[b]["outT"].T.astype(np.float32)
                  + res.results[b + B]["outT"].T.astype(np.float32)
                  + b_out)
    return out


# revision 50
# speedup vs baseline: 1.0076x; 1.0076x over previous
"""Causal multi-head attention block (B=4,S=2048,E=1024,H=16,D=64) on 8 trn2 cores.

Sharding: 4 batches x 2 head-groups (8 heads each) = 8 cores.
Each core: QKV projection for its (batch, head-group), causal attention,
partial output projection over its heads. Host sums the 2 partials per batch
(the "all-reduce after project_out" done at gather time) and adds b_out.

Layout: everything is computed transposed; no on-chip transposes anywhere.
  qkv^T[f, s] = W^T x^T   via matmul(lhsT=W[e,f], rhs=xT[e,s])
  V natural [s, f]        via matmul(lhsT=xT[e,s], rhs=Wv[e,f])
  scores^T[k, q] = K Q^T  via matmul(lhsT=KT[d,k], rhs=QT[d,q]) per head (d=64)
  softmax over k (= partition dim): exp on ACT (scale=1/sqrt(D) fused), the
  denominator comes free from a ones-column appended to V in the AV matmul,
  divide via DVE reciprocal + GpSimd partition_broadcast.
  ans^T[d, q]             via matmul(lhsT=[V|1][k, d+1], rhs=w^T[k, q])
  out^T[e, q] partial     via matmul(lhsT=Wout[f,e], rhs=ansT[f,q])

Projections run in compensated fp8 (e4m3) with DoubleRow perf mode: operands
are pre-scaled by powers of 2 (x*8, W*64) and split hi = fp8(a), lo =
fp8(a - hi); x@W ~ xh@Wh + xh@Wl + xl@Wh, three DoubleRow matmuls per
e-chunk-pair, each contracting 2x128 rows at 0.5 cycles/row -- 4x the bf16
FLOP rate, so the whole projection costs 0.75x its bf16 time at bf16-level
accuracy (residual quantization error ~0.1%). The 1/512 scale is folded into
the psum-drain tensor_scalar ops. SBUF tiles hold (hi_e0, hi_e1, lo_e0,
lo_e1) slot quads so all three matmuls address [p, 2, *] views of one tile.
The output projection does the same with ans split hi/lo at the softmax
divide (ans*8, W_out*64, 6 DoubleRow matmuls per out tile).

Scores for heads 0-3 run in plain fp8 DoubleRow (q/k pre-scaled by 8,
K8/Q8 stored as [32*h + d%32, dhalf, s] via a host-side Wq/Wk column
permutation; exp scale absorbs the 64x) -- 2x the bf16 rate at a measured
~1.4e-2 relative-error cost that fits the 2e-2 budget because each output
element mixes all 16 heads through the output projection (error scales as
sqrt(fp8-head fraction)). Heads 4-7 and all AV matmuls stay bf16: fp8
softmax weights/values cost 2-4e-2 (over budget) for only 2x.

All bf16 matmul operands run the PE at full rate even for narrow (<256)
outputs, so diagonal-band tiles use exact widths, and all DMA traffic
halves vs f32. Inputs are converted to bf16/fp8 on the host.

DMA strategy: every load is one batched transfer ([128, *] tiles built
with rearranges of the DRAM source), issued at kernel start across all
three issue paths (SP/ACT hwdge + Pool swdge); weights and all four x
blocks are SBUF-resident for the whole kernel. Block 0's x/wq/wk/wv are
split into 4 stripes each (one per e-chunk-pair, in separate tiles, so
dependency tracking is per-stripe) and block 0's projection runs 4 psum
groups wide with the pair-loop inner, consuming stripes as they land at
~the DMA supply rate. A short burst of dummy matmuls burns the PE p-state
ramp while the first stripes are in flight. Only output stores (batched in
pairs of e-tiles) remain inside the main loop.

Causality: k-tiles above the diagonal are skipped; diagonal-band tiles use
exact-width matmuls/exp (columns >= j*128) plus a [128,128] triangle mask.

The head-pair loop is software-pipelined three tiles deep, and across
q-block boundaries five tiles deep: the next pair's (or next block's pair
0's) first score/exp tiles are emitted before the current pair's AV drain
and epilogue, so the ACT engine (whose exp backlog gates the final divide
chain) never starves at pair or block boundaries.

Scheduling: the attention inner loop is ACT(exp)-limited while projections
are pure PE work, so projection/output-projection generators are interleaved
(paced round-robin) into each attention block's instruction stream to keep
the in-order PE engine saturated. The final block's output projection is
split so its tail executes during the last softmax epilogue's divide chain.
"""

import numpy as np

B, S, E, H, D = 4, 2048, 1024, 16, 64
NCORES = 8
HG = 2                 # head groups (tensor parallel)
HC = H // HG           # 8 heads per core
FQ = HC * D            # 512 local features per q/k/v
P, NB = 128, 512       # partition tile, free-dim block
ET, ST, KTN, FT = E // P, S // NB, S // P, FQ // P   # 8, 4, 16, 4
EP = ET // 2           # e-chunk pairs (4)
SX, SW = 8.0, 64.0     # fp8 pre-scales for x / weights
INV = 1.0 / (SX * SW)  # 1/512 drain scale

_cache = {}
FACS = {0: 1.20, 1: 1.20, 2: 1.00, 3: 1.05}  # filler pacing per q-block
NSC = 4                # heads with fp8 DoubleRow score matmuls (0 or 4)
SQ = 8.0               # q/k fp8 pre-scale for fp8-score heads
XC1, XC2 = 8, 14       # cross-block seed caps (pend-drain / epilogue)
XC3 = 6                # cross-block seed cap for next block's pair 1
HOIST = 3              # next-pair hoist depth


def _build():
    from contextlib import ExitStack
    import concourse.tile as tile
    import concourse.mybir as mybir
    from concourse import bacc

    dt = mybir.dt
    f32, bf16, f8 = dt.float32, dt.bfloat16, dt.float8e4
    AF = mybir.ActivationFunctionType
    ALU = mybir.AluOpType
    DR = mybir.MatmulPerfMode.DoubleRow
    SCALE = 0.125  # 1/sqrt(D)

    nc = bacc.Bacc("TRN2", target_bir_lowering=False, debug=False,
                   num_devices=NCORES)

    # host-packed fp8 hi/lo slot layouts (contiguous DMA images):
    # xsd[s] = block-0 stripe s [P, 4, NB]; xad[i] = block 1+i [P, 16*NB]
    xsd = nc.dram_tensor("xsd", [EP, P, 4 * NB], f8,
                         kind="ExternalInput").ap()
    xad = nc.dram_tensor("xad", [ST - 1, P, 16 * NB], f8,
                         kind="ExternalInput").ap()
    wqd = nc.dram_tensor("wqd", [EP, P, 4 * FQ], f8,
                         kind="ExternalInput").ap()
    wkd = nc.dram_tensor("wkd", [EP, P, 4 * FQ], f8,
                         kind="ExternalInput").ap()
    wvd = nc.dram_tensor("wvd", [EP, P, 4 * FQ], f8,
                         kind="ExternalInput").ap()
    wod = nc.dram_tensor("wod", [2, P, 4 * E], f8,
                         kind="ExternalInput").ap()
    msk = nc.dram_tensor("msk", [P, P], bf16, kind="ExternalInput").ap()
    bq = nc.dram_tensor("bq", [FQ], f32, kind="ExternalInput").ap()
    bk = nc.dram_tensor("bk", [FQ], f32, kind="ExternalInput").ap()
    bvb = nc.dram_tensor("bvb", [P, FQ], f32, kind="ExternalInput").ap()
    outT = nc.dram_tensor("outT", [E, S], bf16, kind="ExternalOutput").ap()

    with tile.TileContext(nc) as tc:
        with ExitStack() as ctx:
            pers = ctx.enter_context(tc.tile_pool(name="pers", bufs=1))
            pqts = ctx.enter_context(tc.tile_pool(name="pqts", bufs=2))
            pwe = ctx.enter_context(tc.tile_pool(name="pwe", bufs=24))
            pans = ctx.enter_context(tc.tile_pool(name="pans", bufs=4))
            pepi = ctx.enter_context(tc.tile_pool(name="pepi", bufs=4))
            pout = ctx.enter_context(tc.tile_pool(name="pout", bufs=4))
            ps1 = ctx.enter_context(
                tc.tile_pool(name="ps1", bufs=2, space="PSUM"))
            sps = ctx.enter_context(
                tc.tile_pool(name="sps", bufs=2, space="PSUM"))
            avps = ctx.enter_context(
                tc.tile_pool(name="avps", bufs=2, space="PSUM"))

            # ---- resident tensors -------------------------------------
            # heads 0-3 keep K in the fp8 DoubleRow layout K8[p, dhalf, k]
            # with p = 32*h + d%32 (host reorders Wq/Wk columns to match);
            # heads 4-7 keep the bf16 [d, k] layout.
            K8 = pers.tile([P, 2 * S], f8, tag="k8", name="k8")
            KT = [None, None] + [
                pers.tile([P, S], bf16, tag=f"kt{i}", name=f"kt{i}")
                for i in range(2, FT)]
            Vp = [pers.tile([P, HC * (D + 1)], bf16, tag=f"vp{i}",
                            name=f"vp{i}") for i in range(KTN)]
            # x blocks 1..3: [P, pair(4) x slot(4) x NB] fp8, slot order
            # (hi_e0, hi_e1, lo_e0, lo_e1) per chunk pair
            XA = [None] + [pers.tile([P, 16 * NB], f8, tag=f"xa{i}",
                                     name=f"xa{i}") for i in range(1, ST)]
            # block-0 stripes: stripe s = chunk pair (2s, 2s+1)
            XS = [pers.tile([P, 4 * NB], f8, tag=f"xs{i}", name=f"xs{i}")
                  for i in range(EP)]
            WQS = [pers.tile([P, 4 * FQ], f8, tag=f"wqs{i}", name=f"wqs{i}")
                   for i in range(EP)]
            WKS = [pers.tile([P, 4 * FQ], f8, tag=f"wks{i}", name=f"wks{i}")
                   for i in range(EP)]
            WVS = [pers.tile([P, 4 * FQ], f8, tag=f"wvs{i}", name=f"wvs{i}")
                   for i in range(EP)]
            # out-projection weights: j = f-chunk pair (2j, 2j+1)
            WOA = [pers.tile([P, 4 * E], f8, tag=f"woa{j}", name=f"woa{j}")
                   for j in range(2)]

            def _xv(sb, a):
                """[p, 4, NB] slot view of chunk pair a in s-block sb."""
                if sb == 0:
                    return XS[a][:].rearrange("p (t s) -> p t s", s=NB)
                v = XA[sb][:].rearrange("p (a t s) -> p a t s", t=4, s=NB)
                return v[:, a]

            def _wv(W, a):
                """[p, 4, FQ] slot view of weight chunk pair a."""
                return W[a][:].rearrange("p (t f) -> p t f", f=FQ)

            bqt = pers.tile([P, FT], f32, tag="bqt")
            bkt = pers.tile([P, FT], f32, tag="bkt")
            bvt = pers.tile([P, FQ], f32, tag="bvt")
            onesf = pers.tile([P, HC], bf16, tag="onesf")
            mtri = pers.tile([P, P], bf16, tag="mtri")
            dum = pers.tile([P, NB], bf16, tag="dum")
            obt = [pers.tile([P, NB], bf16, tag=f"obt{i}", name=f"obt{i}")
                   for i in range(2)]
            nc.vector.memset(dum[:], 1.0)
            nc.vector.memset(onesf[:], 1.0)

            # ---- startup DMA plan -------------------------------------
            # 4 stripes each for block-0 x / wq / wk / wv (so the first
            # projection matmuls start supply-paced ~3us in), one batched
            # transfer for everything else. Queues: SP=x,
            # ACT=wq+biases+mask, Pool-SWDGE=wk+wv+wo.
            for s in range(EP):
                nc.sync.dma_start(XS[s][:], xsd[s])
                nc.scalar.dma_start(WQS[s][:], wqd[s])
            # small tiles go through SWDGE first so their transfers slot in
            # between the early x/wq stripes without head-of-line blocking
            nc.gpsimd.dma_start(bqt[:], bq.rearrange("(a p) -> p a", p=P))
            nc.gpsimd.dma_start(bkt[:], bk.rearrange("(a p) -> p a", p=P))
            for s in range(EP):
                nc.gpsimd.dma_start(WKS[s][:], wkd[s])
                (nc.scalar if s % 2 else nc.sync).dma_start(
                    WVS[s][:], wvd[s])
            nc.gpsimd.dma_start(mtri[:], msk[:])
            nc.gpsimd.dma_start(bvt[:], bvb[:])
            for sb in range(1, ST):
                nc.sync.dma_start(XA[sb][:], xad[sb - 1])
            for j in range(2):
                nc.gpsimd.dma_start(WOA[j][:], wod[j])

            # per-block state shared between generators
            QTS = {}    # sb -> [q8, q8, qt2, qt3] (hp-indexed)
            ATS = {}    # qb -> [2 tiles: [P, 4, NB] (hi2j, hi2j+1, lo..)]
            XSEED = []  # cross-block hoisted score tiles (next qb, pair 0)
            XSEED2 = []  # same for next block's pair 1

            def _drain_q(sb, ft, ps):
                if ft < 2:
                    if ft == 0:
                        q8 = pqts.tile([P, 2 * NB], f8, tag="q8",
                                       name=f"q8_{sb}")
                        QTS.setdefault(sb, []).append(q8)
                    else:
                        q8 = QTS[sb][0]
                        QTS[sb].append(q8)
                    v = q8[:].rearrange("p (t s) -> p t s", s=NB)
                    nc.vector.tensor_scalar(
                        v[:, ft], ps[:], INV * SQ, bqt[:, ft:ft + 1],
                        op0=ALU.mult, op1=ALU.add)
                else:
                    qt = pqts.tile([P, NB], bf16, tag=f"qts{ft}",
                                   name=f"qts{ft}_{sb}")
                    nc.vector.tensor_scalar(
                        qt[:], ps[:], INV, bqt[:, ft:ft + 1],
                        op0=ALU.mult, op1=ALU.add)
                    QTS.setdefault(sb, []).append(qt)

            def _drain_k(sb, ft, ps):
                cols = slice(sb * NB, (sb + 1) * NB)
                if ft < 2:
                    v = K8[:].rearrange("p (t k) -> p t k", k=S)
                    nc.vector.tensor_scalar(
                        v[:, ft, cols], ps[:], INV * SQ,
                        bkt[:, ft:ft + 1], op0=ALU.mult, op1=ALU.add)
                else:
                    nc.vector.tensor_scalar(
                        KT[ft][:, cols], ps[:], INV, bkt[:, ft:ft + 1],
                        op0=ALU.mult, op1=ALU.add)

            def emit3(ps, wvw, xvw, cols, first, last):
                """3 comp DoubleRow matmuls for one chunk pair into ps."""
                nc.tensor.matmul(ps, wvw[:, 0:2, cols[0]:cols[1]],
                                 xvw[:, 0:2, :], start=first, stop=False,
                                 perf_mode=DR)
                nc.tensor.matmul(ps, wvw[:, 2:4, cols[0]:cols[1]],
                                 xvw[:, 0:2, :], start=False, stop=False,
                                 perf_mode=DR)
                nc.tensor.matmul(ps, wvw[:, 0:2, cols[0]:cols[1]],
                                 xvw[:, 2:4, :], start=False, stop=last,
                                 perf_mode=DR)

            def emit3v(ps, xvw, wvw, stl, first, last):
                """3 comp DoubleRow matmuls, V orientation (x stationary)."""
                c0, c1 = stl * P, (stl + 1) * P
                nc.tensor.matmul(ps, xvw[:, 0:2, c0:c1], wvw[:, 0:2, :],
                                 start=first, stop=False, perf_mode=DR)
                nc.tensor.matmul(ps, xvw[:, 0:2, c0:c1], wvw[:, 2:4, :],
                                 start=False, stop=False, perf_mode=DR)
                nc.tensor.matmul(ps, xvw[:, 2:4, c0:c1], wvw[:, 0:2, :],
                                 start=False, stop=last, perf_mode=DR)

            def proj0():
                """QKV projection of s-block 0, emitted standalone before
                the main loop. Runs 4 psum groups wide (ps1 + borrowed
                score-psum banks, idle until attention starts) so every
                arriving x/w DMA stripe is consumed with 12 matmuls
                (~1.28us) -- close to the ~1.46us/stripe supply rate, so
                the PE tracks the DMA stream with no re-read passes."""
                POOL6 = [(ps1, "ps"), (ps1, "ps"), (sps, "sp"),
                         (sps, "sp"), (avps, "av"), (avps, "av")]
                qoff = [0]

                def quad():
                    off = qoff[0]
                    qoff[0] = (off + 4) % 6
                    return [POOL6[(off + k) % 6][0].tile(
                                [P, NB], f32, tag=POOL6[(off + k) % 6][1],
                                name=f"p0_{off}_{k}")
                            for k in range(4)]
                for wts, dst in ((WQS, "q"), (WKS, "k")):
                    ps = quad()
                    for a in range(EP):
                        wvw = _wv(wts, a)
                        xvw = _xv(0, a)
                        for ft in range(FT):
                            emit3(ps[ft][:], wvw, xvw,
                                  (ft * P, (ft + 1) * P),
                                  first=(a == 0), last=(a == EP - 1))
                    for ft in range(FT):
                        if dst == "q":
                            _drain_q(0, ft, ps[ft])
                        else:
                            _drain_k(0, ft, ps[ft])
                ps = quad()
                for a in range(EP):
                    wvw = _wv(WVS, a)
                    xvw = _xv(0, a)
                    for stl in range(ST):
                        emit3v(ps[stl][:], xvw, wvw, stl,
                               first=(a == 0), last=(a == EP - 1))
                for stl in range(ST):
                    _vp_write(stl, ps[stl])

            def _vp_write(st, ps):
                vview = Vp[st][:].rearrange("p (h c) -> p h c", c=D + 1)
                nc.vector.tensor_copy(
                    vview[:, :, D:D + 1],
                    onesf[:].rearrange("p (h c) -> p h c", c=1))
                nc.vector.scalar_tensor_tensor(
                    vview[:, :, 0:D], ps[:], INV,
                    bvt[:].rearrange("p (h d) -> p h d", d=D),
                    op0=ALU.mult, op1=ALU.add)

            def proj_gen(sb):
                """QKV projection of s-block sb>=1 (all inputs resident).
                Yields between PE chunks; single open psum at a time so the
                shared ps1 ring stays safe under filler interleaving."""
                for ft in range(FT):
                    ps = ps1.tile([P, NB], f32, tag="ps", name=f"psq{ft}_{sb}")
                    for a in range(EP):
                        emit3(ps[:], _wv(WQS, a), _xv(sb, a),
                              (ft * P, (ft + 1) * P),
                              first=(a == 0), last=(a == EP - 1))
                        if a == 1:
                            yield
                    _drain_q(sb, ft, ps)
                    yield
                for ft in range(FT):
                    ps = ps1.tile([P, NB], f32, tag="ps", name=f"psk{ft}_{sb}")
                    for a in range(EP):
                        emit3(ps[:], _wv(WKS, a), _xv(sb, a),
                              (ft * P, (ft + 1) * P),
                              first=(a == 0), last=(a == EP - 1))
                        if a == 1:
                            yield
                    _drain_k(sb, ft, ps)
                    yield
                for stl in range(ST):
                    ps = ps1.tile([P, NB], f32, tag="ps",
                                  name=f"psv{stl}_{sb}")
                    for a in range(EP):
                        emit3v(ps[:], _xv(sb, a), _wv(WVS, a), stl,
                               first=(a == 0), last=(a == EP - 1))
                        if a == 1:
                            yield
                    _vp_write(ST * sb + stl, ps)
                    yield

            def attn_gen(qb):
                """Attention for q-block qb. Yields once per kt step.

                The head-pair loop is software-pipelined: the NEXT pair's
                first score/exp tile is emitted before this pair's AV drain
                and epilogue, so the ACT engine never starves at pair
                boundaries (its backlog gates the final divide chain)."""
                nkt = ST * (qb + 1)
                ATS[qb] = [pans.tile([P, 4 * NB], f8, tag=f"at2{j}",
                                     name=f"at2{j}_{qb}") for j in range(2)]

                def tile_step(hp, kt, qb2=qb):
                    QT2 = QTS[qb2]
                    j = kt - ST * qb2
                    c0 = j * P if j >= 0 else 0
                    # both heads of the pair share one 2-bank psum tile
                    # and a single strided exp call
                    sp = sps.tile([P, 2 * NB], f32, tag="sp",
                                  name=f"sp{qb2}_{hp}_{kt}")
                    if hp < 2:
                        # fp8 DoubleRow scores: one matmul per head,
                        # contracting both d-halves at 0.5 cycles/row
                        k8v = K8[:].rearrange("p (t k) -> p t k", k=S)
                        q8v = QT2[0][:].rearrange("p (t s) -> p t s", s=NB)
                        for i in range(2):
                            h = 2 * hp + i
                            hs = slice(h * 32, (h + 1) * 32)
                            nc.tensor.matmul(
                                sp[:, i * NB + c0:(i + 1) * NB],
                                k8v[hs, :, kt * P:(kt + 1) * P],
                                q8v[hs, :, c0:NB],
                                start=True, stop=True, perf_mode=DR,
                                tile_position=(h * 32, 0))
                    else:
                        for i in range(2):
                            nc.tensor.matmul(
                                sp[:, i * NB + c0:(i + 1) * NB],
                                KT[hp][i * D:(i + 1) * D,
                                       kt * P:(kt + 1) * P],
                                QT2[hp][i * D:(i + 1) * D, c0:NB],
                                start=True, stop=True)
                    w = pwe.tile([P, 2 * NB], bf16, tag="w",
                                 name=f"w{qb2}_{hp}_{kt}")
                    spv = sp[:].rearrange("p (h q) -> p h q", h=2)
                    wv_ = w[:].rearrange("p (h q) -> p h q", h=2)
                    nc.scalar.activation(wv_[:, :, c0:NB],
                                         spv[:, :, c0:NB],
                                         AF.Exp,
                                         scale=(SCALE / (SQ * SQ)
                                                if hp < 2 else SCALE))
                    if j >= 0:
                        nc.vector.tensor_mul(
                            wv_[:, :, c0:c0 + P], wv_[:, :, c0:c0 + P],
                            mtri[:]
                            .rearrange("p (a q) -> p a q", a=1)
                            .broadcast_to([P, 2, P]))
                    return (kt, c0, w)

                hoist = list(XSEED)
                del XSEED[:]
                for hp in range(FT):
                    av = [avps.tile([D + 1, NB], f32, tag="av",
                                    name=f"av{qb}_{hp}_{i}")
                          for i in range(2)]

                    def emit_av(ent, last, av=av, hp=hp):
                        k0, pc0, w0 = ent
                        for i in range(2):
                            nc.tensor.matmul(
                                av[i][:, pc0:NB],
                                Vp[k0][:, (2 * hp + i) * (D + 1):
                                                (2 * hp + i + 1) * (D + 1)],
                                w0[:, i * NB + pc0:(i + 1) * NB],
                                start=(k0 == 0), stop=last)

                    pend = list(hoist)
                    ktlo = len(hoist)
                    hoist = []
                    if hp == 0 and XSEED2:
                        hoist = list(XSEED2)
                        del XSEED2[:]
                    for kt in range(ktlo, nkt):
                        pend.append(tile_step(hp, kt))
                        if len(pend) > 2:
                            emit_av(pend.pop(0), last=False)
                        if kt == nkt - 1:
                            if hp + 1 < FT:
                                hoist.append(
                                    tile_step(hp + 1, len(hoist)))
                            elif (qb + 1 < ST
                                  and len(QTS.get(qb + 1, [])) == FT):
                                XSEED.append(tile_step(0, 0, qb + 1))
                        yield
                    while pend:
                        ent = pend.pop(0)
                        emit_av(ent, last=not pend)
                        if 0 < len(hoist) < min(HOIST, nkt) \
                                and hp + 1 < FT:
                            hoist.append(tile_step(hp + 1, len(hoist)))
                        elif (hp + 1 == FT and qb + 1 < ST
                              and len(QTS.get(qb + 1, [])) == FT):
                            nx = min(XC1, ST * (qb + 2))
                            for _x in range(3):
                                if 0 < len(XSEED) < nx:
                                    XSEED.append(
                                        tile_step(0, len(XSEED), qb + 1))
                        yield
                    # epilogue per head: tmp = copy(av) (frees the psum
                    # slot fast, no divide-chain wait), then in-place
                    # tmp = (tmp*8)/Z; at_hi = fp8(tmp), at_lo =
                    # fp8(tmp - at_hi) rounded pair-wide below.
                    tmpb = pepi.tile([P, NB], bf16, tag="tmpb",
                                     name=f"tmpb{qb}_{hp}")
                    # the very last pair's chain is the exposed tail: route
                    # its copies through ACT (idle after the final exp) and
                    # round hi/lo per head so head 0's rounding overlaps
                    # head 1's divide chain.
                    last_pair = (qb == ST - 1 and hp == FT - 1)
                    on_act = qb <= 1 or last_pair
                    j, jj = hp // 2, hp % 2
                    atv = ATS[qb][j][:].rearrange("p (t s) -> p t s", s=NB)
                    for i in range(2):
                        se = pepi.tile([1, NB], f32, tag="se",
                                       name=f"se{qb}_{hp}_{i}")
                        if on_act:
                            nc.scalar.copy(se[:], av[i][D:D + 1, :])
                            nc.scalar.copy(tmpb[i * D:(i + 1) * D, :],
                                           av[i][0:D, :])
                        else:
                            nc.vector.tensor_copy(se[:], av[i][D:D + 1, :])
                            nc.vector.tensor_copy(
                                tmpb[i * D:(i + 1) * D, :], av[i][0:D, :])
                        nc.vector.reciprocal_approx_fast(se[:], se[:])
                        bch = pepi.tile([P, NB], f32, tag="bch",
                                        name=f"bch{qb}_{hp}_{i}")
                        nc.gpsimd.partition_broadcast(
                            bch[0:(i + 1) * D, :], se[:],
                            channels=(i + 1) * D)
                        nc.vector.scalar_tensor_tensor(
                            tmpb[i * D:(i + 1) * D, :],
                            tmpb[i * D:(i + 1) * D, :], SX,
                            bch[i * D:(i + 1) * D, :],
                            op0=ALU.mult, op1=ALU.mult)
                        if last_pair:
                            hs = slice(i * D, (i + 1) * D)
                            nc.vector.tensor_copy(atv[hs, jj], tmpb[hs, :])
                            nc.vector.tensor_tensor(
                                atv[hs, 2 + jj], tmpb[hs, :], atv[hs, jj],
                                op=ALU.subtract)
                        if (hp + 2 >= FT and qb + 1 < ST
                                and len(QTS.get(qb + 1, [])) == FT):
                            nx = min(XC2, ST * (qb + 2))
                            for _x in range(4):
                                if len(XSEED) < nx \
                                        and (XSEED or hp + 1 == FT):
                                    XSEED.append(
                                        tile_step(0, len(XSEED), qb + 1))
                                elif (len(XSEED) >= nx
                                      and len(XSEED2) < min(
                                          XC3, ST * (qb + 2))):
                                    XSEED2.append(
                                        tile_step(1, len(XSEED2), qb + 1))
                        yield
                    if not last_pair:
                        # pair-wide hi/lo rounding (both heads at once)
                        nc.vector.tensor_copy(atv[:, jj], tmpb[:])
                        nc.vector.tensor_tensor(
                            atv[:, 2 + jj], tmpb[:], atv[:, jj],
                            op=ALU.subtract)
                    yield

            def store_pair(qb, et, ob, eng=None):
                # all loads are issued up-front, so SP.SEQ is free during
                # the main loop; SWDGE stores would block Pool.SEQ (and the
                # softmax broadcasts) while waiting for staging data
                (eng or nc.sync).dma_start(
                    outT[(et - 1) * P:(et + 1) * P,
                         qb * NB:(qb + 1) * NB]
                    .rearrange("(a p) s -> p a s", p=P),
                    ob[:].rearrange("p (a s) -> p a s", s=NB))

            def out_mm(po, qb, et, jset=(0, 1), first=True, last=True):
                """comp DoubleRow out-projection matmuls for e-tile et."""
                for jx, j in enumerate(jset):
                    wvw = WOA[j][:].rearrange("p (t e) -> p t e", e=E)
                    atv = ATS[qb][j][:].rearrange("p (t s) -> p t s", s=NB)
                    c = (et * P, (et + 1) * P)
                    nc.tensor.matmul(
                        po, wvw[:, 0:2, c[0]:c[1]], atv[:, 0:2, :],
                        start=(first and jx == 0), stop=False, perf_mode=DR)
                    nc.tensor.matmul(
                        po, wvw[:, 2:4, c[0]:c[1]], atv[:, 0:2, :],
                        start=False, stop=False, perf_mode=DR)
                    nc.tensor.matmul(
                        po, wvw[:, 0:2, c[0]:c[1]], atv[:, 2:4, :],
                        start=False,
                        stop=(last and jx == len(jset) - 1), perf_mode=DR)

            def out_gen(qb, ets=None, act_copy=False):
                """Output projection of q-block qb. Yields per e-tile.
                Stores are batched in pairs of e-tiles. act_copy routes the
                psum drains through ACT (for tail portions emitted after the
                last exp, when ACT is idle but DVE is still busy)."""
                ob = None
                for et in (range(ET) if ets is None else ets):
                    if et % 2 == 0:
                        ob = pout.tile([P, 2 * NB], bf16, tag="ob",
                                       name=f"ob{qb}_{et}")
                    po = ps1.tile([P, NB], f32, tag="ps",
                                  name=f"po{qb}_{et}")
                    out_mm(po[:], qb, et)
                    if act_copy:
                        nc.scalar.mul(
                            ob[:, (et % 2) * NB:(et % 2 + 1) * NB], po[:],
                            INV)
                    else:
                        nc.vector.tensor_scalar(
                            ob[:, (et % 2) * NB:(et % 2 + 1) * NB], po[:],
                            INV, None, op0=ALU.mult)
                    if et % 2 == 1:
                        store_pair(qb, et, ob)
                    yield

            O3 = {}

            def out3_a():
                """Final-block e-tiles 0-5 open with the j=0 (head pairs
                0-1) halves: pure PE work depending only on those pairs.
                Emitted right after the last pair's AV drain so it executes
                during that pair's divide chain. The open groups borrow
                attention's score psum slots."""
                for et in (0, 1, 2, 3, 4, 5):
                    pool, tg = ((ps1, "ps") if et < 2 else
                                (sps, "sp") if et < 4 else (avps, "av"))
                    po = pool.tile([P, NB], f32, tag=tg, name=f"po3a_{et}")
                    O3[et] = po[:]
                    out_mm(po[:], ST - 1, et, jset=(0,), first=True,
                           last=False)

            def out3():
                """Final block: j=1 closers for e-tiles 0-5, full
                accumulations for e-tiles 6-7, stores batched in pairs with
                single-tile tail stores on alternating queues."""
                qb = ST - 1
                out3_a()
                pos = O3
                ob = None
                for et in range(ET):
                    if et < 6:
                        po = pos[et]
                        out_mm(po, qb, et, jset=(1,), first=False,
                               last=True)
                    else:
                        pool, tg = (ps1, "ps") if et == 6 else (sps, "sp")
                        po = pool.tile([P, NB], f32, tag=tg,
                                       name=f"po3b_{et}")[:]
                        out_mm(po, qb, et)
                    if et < 6:
                        if et % 2 == 0:
                            ob = pout.tile([P, 2 * NB], bf16, tag="ob",
                                           name=f"ob{qb}_{et}")
                            nc.scalar.mul(ob[:, 0:NB], po, INV)
                        else:
                            nc.vector.tensor_scalar(
                                ob[:, NB:2 * NB], po, INV, None,
                                op0=ALU.mult)
                            store_pair(qb, et, ob)
                    else:
                        # drain tail: single-tile stores on alternating
                        # queues so the last transfers issue immediately
                        ob = obt[et - 6]
                        if et == 6:
                            nc.scalar.mul(ob[:], po, INV)
                        else:
                            nc.vector.tensor_scalar(
                                ob[:], po, INV, None, op0=ALU.mult)
                        (nc.gpsimd if et == 6 else nc.sync).dma_start(
                            outT[et * P:(et + 1) * P,
                                 qb * NB:(qb + 1) * NB], ob[:])

            def drain(g):
                for _ in g:
                    pass

            # warmup: burn the PE p-state ramp while the first input
            # stripes are still in flight, so real matmuls start full-rate
            for i in range(4):
                dp = avps.tile([8, NB], f32, tag="av", name=f"dummy{i}")
                nc.tensor.matmul(dp[:], dum[:, 0:8], dum[:],
                                 start=True, stop=True)
            proj0()
            # Filler plan: spread PE-only work over each attention block to
            # absorb the ACT(exp) deficit; OUT(1)/OUT(2) go to attention(3),
            # which has no projection work left to hide exp latency.
            plans = {
                0: ([lambda: proj_gen(1)], 24),
                1: ([lambda: proj_gen(2)], 24),
                2: ([lambda: proj_gen(3)], 24),
                3: ([lambda: out_gen(0), lambda: out_gen(1),
                     lambda: out_gen(2)], 24),
            }
            for qb in range(ST):
                mk, nf = plans[qb]
                fillers = [m() for m in mk]
                na = 4 * (ST * (qb + 1) + 6)
                fac = FACS[qb]
                rate = fac * nf / na
                acc, fi = 0.0, 0
                for _ in attn_gen(qb):
                    acc += rate
                    while acc >= 1.0 and fillers:
                        acc -= 1.0
                        f = fillers[fi % len(fillers)]
                        fi += 1
                        try:
                            next(f)
                        except StopIteration:
                            fillers.remove(f)
                for f in fillers:
                    drain(f)
            out3()
    nc.compile()
    return nc


def _mask_tri():
    import ml_dtypes
    kp = np.arange(P)[:, None]
    qf = np.arange(P)[None, :]
    return (qf >= kp).astype(ml_dtypes.bfloat16)


def _qk_perm():
    """Column permutation for Wq/Wk: features of heads 0-3 reordered to
    (d-half, 32*h + d%32) so projection psums land in the fp8 DoubleRow
    score layout; heads 4-7 unchanged."""
    perm = np.arange(FQ)
    for newcol in range(2 * P):
        ft, p = divmod(newcol, P)
        h, dd = divmod(p, 32)
        perm[newcol] = h * D + ft * 32 + dd
    return perm


def _qk_bias(bvec, perm):
    """bias vector reordered like the W columns, with the fp8-score heads'
    entries pre-scaled by SQ (their drains fold q8 = SQ*(q + bias))."""
    b = np.ascontiguousarray(bvec.reshape(FQ)[perm])
    b[:2 * P] *= SQ
    return b


def _hilo(a):
    """fp8 hi/lo split: a ~ hi + lo, both e4m3."""
    import ml_dtypes
    f8 = ml_dtypes.float8_e4m3fn
    a = np.ascontiguousarray(a, dtype=np.float32)
    hi = a.astype(f8)
    lo = (a - hi.astype(np.float32)).astype(f8)
    return hi, lo


def _slotpack(hi, lo, cols):
    """[rows=2*P, cols] hi/lo planes -> [P, 4, cols] slot quad
    (hi_e0, hi_e1, lo_e0, lo_e1)."""
    out = np.empty((P, 4, cols), dtype=hi.dtype)
    out[:, 0] = hi[0:P]
    out[:, 1] = hi[P:2 * P]
    out[:, 2] = lo[0:P]
    out[:, 3] = lo[P:2 * P]
    return out


def _pack_w(w):
    """[E or FQ, cols] scaled weight -> [npairs, P, 4*cols] stripe images."""
    hi, lo = _hilo(w)
    n = w.shape[0] // (2 * P)
    return np.stack([
        _slotpack(hi[2 * s * P:(2 * s + 2) * P],
                  lo[2 * s * P:(2 * s + 2) * P],
                  w.shape[1]).reshape(P, -1)
        for s in range(n)])


def _pack_x(xT):
    """[E, S] scaled x^T -> (stripe images [EP, P, 4*NB],
    block images [ST-1, P, 16*NB])."""
    hi, lo = _hilo(xT)
    xs = np.stack([
        _slotpack(hi[2 * s * P:(2 * s + 2) * P, 0:NB],
                  lo[2 * s * P:(2 * s + 2) * P, 0:NB], NB).reshape(P, -1)
        for s in range(EP)])
    xa = np.empty((ST - 1, P, 16 * NB), dtype=hi.dtype)
    for sb in range(1, ST):
        c = slice(sb * NB, (sb + 1) * NB)
        blk = np.stack([
            _slotpack(hi[2 * a * P:(2 * a + 2) * P, c],
                      lo[2 * a * P:(2 * a + 2) * P, c], NB)
            for a in range(EP)], axis=1)          # [P, 4, 4, NB]
        xa[sb - 1] = blk.reshape(P, -1)
    return xs, xa


def kernel(x, W_qkv, b_qkv, W_out, b_out):
    from concourse.bass_utils import run_bass_kernel_spmd

    if "nc" not in _cache:
        _cache["nc"] = _build()
    nc = _cache["nc"]

    x = np.asarray(x, dtype=np.float32)
    W_qkv = np.asarray(W_qkv, dtype=np.float32)
    b_qkv = np.asarray(b_qkv, dtype=np.float32)
    W_out = np.asarray(W_out, dtype=np.float32)
    b_out = np.asarray(b_out, dtype=np.float32)

    mtri = _mask_tri()
    perm = _qk_perm()
    in_maps = []
    for c in range(NCORES):
        b, g = c % B, c // B
        hs = slice(g * HC, (g + 1) * HC)
        Wl = W_qkv[:, :, hs, :]                       # [E, 3, HC, D]
        xs_im, xa_im = _pack_x(x[b].T * SX)
        in_maps.append({
            "xsd": xs_im,
            "xad": xa_im,
            "wqd": _pack_w(Wl[:, 0].reshape(E, FQ)[:, perm] * SW),
            "wkd": _pack_w(Wl[:, 1].reshape(E, FQ)[:, perm] * SW),
            "wvd": _pack_w(Wl[:, 2].reshape(E, FQ) * SW),
            "wod": _pack_w(W_out[hs].reshape(FQ, E) * SW),
            "msk": mtri,
            "bq": _qk_bias(b_qkv[0, hs], perm),
            "bk": _qk_bias(b_qkv[1, hs], perm),
            "bvb": np.broadcast_to(b_qkv[2, hs].reshape(1, FQ),
                                   (P, FQ)).copy(),
        })

    try:
        res = run_bass_kernel_spmd(nc, in_maps, core_ids=list(range(NCORES)))
    except Exception:
        # transient device wedges (NRT_EXEC_UNIT_UNRECOVERABLE) clear on retry
        res = run_bass_kernel_spmd(nc, in_maps, core_ids=list(range(NCORES)))
    _cache["last_results"] = res
    out = np.empty((B, S, E), dtype=np.float32)
    for b in range(B):
        out[b] = (res.results[b]["outT"].T.astype(np.float32)
                  + res.results[b + B]["outT"].T.astype(np.float32)
                  + b_out)
    return out


# revision 53
# speedup vs baseline: 1.0083x; 1.0007x over previous
"""Causal multi-head attention block (B=4,S=2048,E=1024,H=16,D=64) on 8 trn2 cores.

Sharding: 4 batches x 2 head-groups (8 heads each) = 8 cores.
Each core: QKV projection for its (batch, head-group), causal attention,
partial output projection over its heads. Host sums the 2 partials per batch
(the "all-reduce after project_out" done at gather time) and adds b_out.

Layout: everything is computed transposed; no on-chip transposes anywhere.
  qkv^T[f, s] = W^T x^T   via matmul(lhsT=W[e,f], rhs=xT[e,s])
  V natural [s, f]        via matmul(lhsT=xT[e,s], rhs=Wv[e,f])
  scores^T[k, q] = K Q^T  via matmul(lhsT=KT[d,k], rhs=QT[d,q]) per head (d=64)
  softmax over k (= partition dim): exp on ACT (scale=1/sqrt(D) fused), the
  denominator comes free from a ones-column appended to V in the AV matmul,
  divide via DVE reciprocal + GpSimd partition_broadcast.
  ans^T[d, q]             via matmul(lhsT=[V|1][k, d+1], rhs=w^T[k, q])
  out^T[e, q] partial     via matmul(lhsT=Wout[f,e], rhs=ansT[f,q])

Projections run in compensated fp8 (e4m3) with DoubleRow perf mode: operands
are pre-scaled by powers of 2 (x*8, W*64) and split hi = fp8(a), lo =
fp8(a - hi); x@W ~ xh@Wh + xh@Wl + xl@Wh, three DoubleRow matmuls per
e-chunk-pair, each contracting 2x128 rows at 0.5 cycles/row -- 4x the bf16
FLOP rate, so the whole projection costs 0.75x its bf16 time at bf16-level
accuracy (residual quantization error ~0.1%). The 1/512 scale is folded into
the psum-drain tensor_scalar ops. SBUF tiles hold (hi_e0, hi_e1, lo_e0,
lo_e1) slot quads so all three matmuls address [p, 2, *] views of one tile.
The output projection does the same with ans split hi/lo at the softmax
divide (ans*8, W_out*64, 6 DoubleRow matmuls per out tile).

Scores for heads 0-3 run in plain fp8 DoubleRow (q/k pre-scaled by 8,
K8/Q8 stored as [32*h + d%32, dhalf, s] via a host-side Wq/Wk column
permutation; exp scale absorbs the 64x) -- 2x the bf16 rate at a measured
~1.4e-2 relative-error cost that fits the 2e-2 budget because each output
element mixes all 16 heads through the output projection (error scales as
sqrt(fp8-head fraction)). Heads 4-7 and all AV matmuls stay bf16: fp8
softmax weights/values cost 2-4e-2 (over budget) for only 2x.

All bf16 matmul operands run the PE at full rate even for narrow (<256)
outputs, so diagonal-band tiles use exact widths, and all DMA traffic
halves vs f32. Inputs are converted to bf16/fp8 on the host.

DMA strategy: every load is one batched transfer ([128, *] tiles built
with rearranges of the DRAM source), issued at kernel start across all
three issue paths (SP/ACT hwdge + Pool swdge); weights and all four x
blocks are SBUF-resident for the whole kernel. Block 0's x/wq/wk/wv are
split into 4 stripes each (one per e-chunk-pair, in separate tiles, so
dependency tracking is per-stripe) and block 0's projection runs 4 psum
groups wide with the pair-loop inner, consuming stripes as they land at
~the DMA supply rate. A short burst of dummy matmuls burns the PE p-state
ramp while the first stripes are in flight. Only output stores (batched in
pairs of e-tiles) remain inside the main loop.

Causality: k-tiles above the diagonal are skipped; diagonal-band tiles use
exact-width matmuls/exp (columns >= j*128) plus a [128,128] triangle mask.

The head-pair loop is software-pipelined three tiles deep, and across
q-block boundaries five tiles deep: the next pair's (or next block's pair
0's) first score/exp tiles are emitted before the current pair's AV drain
and epilogue, so the ACT engine (whose exp backlog gates the final divide
chain) never starves at pair or block boundaries.

Scheduling: the attention inner loop is ACT(exp)-limited while projections
are pure PE work, so projection/output-projection generators are interleaved
(paced round-robin) into each attention block's instruction stream to keep
the in-order PE engine saturated. The final block's output projection is
split so its tail executes during the last softmax epilogue's divide chain.
"""

import numpy as np

B, S, E, H, D = 4, 2048, 1024, 16, 64
NCORES = 8
HG = 2                 # head groups (tensor parallel)
HC = H // HG           # 8 heads per core
FQ = HC * D            # 512 local features per q/k/v
P, NB = 128, 512       # partition tile, free-dim block
ET, ST, KTN, FT = E // P, S // NB, S // P, FQ // P   # 8, 4, 16, 4
EP = ET // 2           # e-chunk pairs (4)
SX, SW = 8.0, 64.0     # fp8 pre-scales for x / weights
INV = 1.0 / (SX * SW)  # 1/512 drain scale

_cache = {}
FACS = {0: 1.20, 1: 1.20, 2: 1.00, 3: 1.05}  # filler pacing per q-block
NSC = 4                # heads with fp8 DoubleRow score matmuls (0 or 4)
SQ = 8.0               # q/k fp8 pre-scale for fp8-score heads
XC1, XC2 = 8, 14       # cross-block seed caps (pend-drain / epilogue)
XC3 = 6                # cross-block seed cap for next block's pair 1
HOIST = 3              # next-pair hoist depth


def _build():
    from contextlib import ExitStack
    import concourse.tile as tile
    import concourse.mybir as mybir
    from concourse import bacc

    dt = mybir.dt
    f32, bf16, f8 = dt.float32, dt.bfloat16, dt.float8e4
    AF = mybir.ActivationFunctionType
    ALU = mybir.AluOpType
    DR = mybir.MatmulPerfMode.DoubleRow
    SCALE = 0.125  # 1/sqrt(D)

    nc = bacc.Bacc("TRN2", target_bir_lowering=False, debug=False,
                   num_devices=NCORES)

    # host-packed fp8 hi/lo slot layouts (contiguous DMA images):
    # xsd[s] = block-0 stripe s [P, 4, NB]; xad[i] = block 1+i [P, 16*NB]
    xsd = nc.dram_tensor("xsd", [EP, P, 4 * NB], f8,
                         kind="ExternalInput").ap()
    xad = nc.dram_tensor("xad", [ST - 1, P, 16 * NB], f8,
                         kind="ExternalInput").ap()
    wqd = nc.dram_tensor("wqd", [EP, P, 4 * FQ], f8,
                         kind="ExternalInput").ap()
    wkd = nc.dram_tensor("wkd", [EP, P, 4 * FQ], f8,
                         kind="ExternalInput").ap()
    wvd = nc.dram_tensor("wvd", [EP, P, 4 * FQ], f8,
                         kind="ExternalInput").ap()
    wod = nc.dram_tensor("wod", [2, P, 4 * E], f8,
                         kind="ExternalInput").ap()
    msk = nc.dram_tensor("msk", [P, P], bf16, kind="ExternalInput").ap()
    bq = nc.dram_tensor("bq", [FQ], f32, kind="ExternalInput").ap()
    bk = nc.dram_tensor("bk", [FQ], f32, kind="ExternalInput").ap()
    bvb = nc.dram_tensor("bvb", [P, FQ], f32, kind="ExternalInput").ap()
    outT = nc.dram_tensor("outT", [E, S], bf16, kind="ExternalOutput").ap()

    with tile.TileContext(nc) as tc:
        with ExitStack() as ctx:
            pers = ctx.enter_context(tc.tile_pool(name="pers", bufs=1))
            pqts = ctx.enter_context(tc.tile_pool(name="pqts", bufs=2))
            pwe = ctx.enter_context(tc.tile_pool(name="pwe", bufs=24))
            pans = ctx.enter_context(tc.tile_pool(name="pans", bufs=4))
            pepi = ctx.enter_context(tc.tile_pool(name="pepi", bufs=4))
            pout = ctx.enter_context(tc.tile_pool(name="pout", bufs=4))
            ps1 = ctx.enter_context(
                tc.tile_pool(name="ps1", bufs=2, space="PSUM"))
            sps = ctx.enter_context(
                tc.tile_pool(name="sps", bufs=2, space="PSUM"))
            avps = ctx.enter_context(
                tc.tile_pool(name="avps", bufs=2, space="PSUM"))

            # ---- resident tensors -------------------------------------
            # heads 0-3 keep K in the fp8 DoubleRow layout K8[p, dhalf, k]
            # with p = 32*h + d%32 (host reorders Wq/Wk columns to match);
            # heads 4-7 keep the bf16 [d, k] layout.
            K8 = pers.tile([P, 2 * S], f8, tag="k8", name="k8")
            KT = [None, None] + [
                pers.tile([P, S], bf16, tag=f"kt{i}", name=f"kt{i}")
                for i in range(2, FT)]
            Vp = [pers.tile([P, HC * (D + 1)], bf16, tag=f"vp{i}",
                            name=f"vp{i}") for i in range(KTN)]
            # x blocks 1..3: [P, pair(4) x slot(4) x NB] fp8, slot order
            # (hi_e0, hi_e1, lo_e0, lo_e1) per chunk pair
            XA = [None] + [pers.tile([P, 16 * NB], f8, tag=f"xa{i}",
                                     name=f"xa{i}") for i in range(1, ST)]
            # block-0 stripes: stripe s = chunk pair (2s, 2s+1)
            XS = [pers.tile([P, 4 * NB], f8, tag=f"xs{i}", name=f"xs{i}")
                  for i in range(EP)]
            WQS = [pers.tile([P, 4 * FQ], f8, tag=f"wqs{i}", name=f"wqs{i}")
                   for i in range(EP)]
            WKS = [pers.tile([P, 4 * FQ], f8, tag=f"wks{i}", name=f"wks{i}")
                   for i in range(EP)]
            WVS = [pers.tile([P, 4 * FQ], f8, tag=f"wvs{i}", name=f"wvs{i}")
                   for i in range(EP)]
            # out-projection weights: j = f-chunk pair (2j, 2j+1)
            WOA = [pers.tile([P, 4 * E], f8, tag=f"woa{j}", name=f"woa{j}")
                   for j in range(2)]

            def _xv(sb, a):
                """[p, 4, NB] slot view of chunk pair a in s-block sb."""
                if sb == 0:
                    return XS[a][:].rearrange("p (t s) -> p t s", s=NB)
                v = XA[sb][:].rearrange("p (a t s) -> p a t s", t=4, s=NB)
                return v[:, a]

            def _wv(W, a):
                """[p, 4, FQ] slot view of weight chunk pair a."""
                return W[a][:].rearrange("p (t f) -> p t f", f=FQ)

            bqt = pers.tile([P, FT], f32, tag="bqt")
            bkt = pers.tile([P, FT], f32, tag="bkt")
            bvt = pers.tile([P, FQ], f32, tag="bvt")
            onesf = pers.tile([P, HC], bf16, tag="onesf")
            mtri = pers.tile([P, P], bf16, tag="mtri")
            dum = pers.tile([P, NB], bf16, tag="dum")
            obt = [pers.tile([P, NB], bf16, tag=f"obt{i}", name=f"obt{i}")
                   for i in range(2)]
            nc.vector.memset(dum[:], 1.0)
            nc.vector.memset(onesf[:], 1.0)

            # ---- startup DMA plan -------------------------------------
            # 4 stripes each for block-0 x / wq / wk / wv (so the first
            # projection matmuls start supply-paced ~3us in), one batched
            # transfer for everything else. Queues: SP=x,
            # ACT=wq+biases+mask, Pool-SWDGE=wk+wv+wo.
            for s in range(EP):
                nc.sync.dma_start(XS[s][:], xsd[s])
                nc.scalar.dma_start(WQS[s][:], wqd[s])
            # small tiles go through SWDGE first so their transfers slot in
            # between the early x/wq stripes without head-of-line blocking
            nc.gpsimd.dma_start(bqt[:], bq.rearrange("(a p) -> p a", p=P))
            nc.gpsimd.dma_start(bkt[:], bk.rearrange("(a p) -> p a", p=P))
            for s in range(EP):
                nc.gpsimd.dma_start(WKS[s][:], wkd[s])
                (nc.scalar if s % 2 else nc.sync).dma_start(
                    WVS[s][:], wvd[s])
            nc.gpsimd.dma_start(mtri[:], msk[:])
            nc.gpsimd.dma_start(bvt[:], bvb[:])
            for sb in range(1, ST):
                nc.sync.dma_start(XA[sb][:], xad[sb - 1])
            for j in range(2):
                nc.gpsimd.dma_start(WOA[j][:], wod[j])

            # per-block state shared between generators
            QTS = {}    # sb -> [q8, q8, qt2, qt3] (hp-indexed)
            ATS = {}    # qb -> [2 tiles: [P, 4, NB] (hi2j, hi2j+1, lo..)]
            XSEED = []  # cross-block hoisted score tiles (next qb, pair 0)
            XSEED2 = []  # same for next block's pair 1

            def _drain_q(sb, ft, ps):
                if ft < 2:
                    if ft == 0:
                        q8 = pqts.tile([P, 2 * NB], f8, tag="q8",
                                       name=f"q8_{sb}")
                        QTS.setdefault(sb, []).append(q8)
                    else:
                        q8 = QTS[sb][0]
                        QTS[sb].append(q8)
                    v = q8[:].rearrange("p (t s) -> p t s", s=NB)
                    nc.vector.tensor_scalar(
                        v[:, ft], ps[:], INV * SQ, bqt[:, ft:ft + 1],
                        op0=ALU.mult, op1=ALU.add)
                else:
                    qt = pqts.tile([P, NB], bf16, tag=f"qts{ft}",
                                   name=f"qts{ft}_{sb}")
                    nc.vector.tensor_scalar(
                        qt[:], ps[:], INV, bqt[:, ft:ft + 1],
                        op0=ALU.mult, op1=ALU.add)
                    QTS.setdefault(sb, []).append(qt)

            def _drain_k(sb, ft, ps):
                cols = slice(sb * NB, (sb + 1) * NB)
                if ft < 2:
                    v = K8[:].rearrange("p (t k) -> p t k", k=S)
                    nc.vector.tensor_scalar(
                        v[:, ft, cols], ps[:], INV * SQ,
                        bkt[:, ft:ft + 1], op0=ALU.mult, op1=ALU.add)
                else:
                    nc.vector.tensor_scalar(
                        KT[ft][:, cols], ps[:], INV, bkt[:, ft:ft + 1],
                        op0=ALU.mult, op1=ALU.add)

            def emit3(ps, wvw, xvw, cols, first, last):
                """3 comp DoubleRow matmuls for one chunk pair into ps."""
                nc.tensor.matmul(ps, wvw[:, 0:2, cols[0]:cols[1]],
                                 xvw[:, 0:2, :], start=first, stop=False,
                                 perf_mode=DR)
                nc.tensor.matmul(ps, wvw[:, 2:4, cols[0]:cols[1]],
                                 xvw[:, 0:2, :], start=False, stop=False,
                                 perf_mode=DR)
                nc.tensor.matmul(ps, wvw[:, 0:2, cols[0]:cols[1]],
                                 xvw[:, 2:4, :], start=False, stop=last,
                                 perf_mode=DR)

            def emit3v(ps, xvw, wvw, stl, first, last):
                """3 comp DoubleRow matmuls, V orientation (x stationary)."""
                c0, c1 = stl * P, (stl + 1) * P
                nc.tensor.matmul(ps, xvw[:, 0:2, c0:c1], wvw[:, 0:2, :],
                                 start=first, stop=False, perf_mode=DR)
                nc.tensor.matmul(ps, xvw[:, 0:2, c0:c1], wvw[:, 2:4, :],
                                 start=False, stop=False, perf_mode=DR)
                nc.tensor.matmul(ps, xvw[:, 2:4, c0:c1], wvw[:, 0:2, :],
                                 start=False, stop=last, perf_mode=DR)

            def proj0():
                """QKV projection of s-block 0, emitted standalone before
                the main loop. Runs 4 psum groups wide (ps1 + borrowed
                score-psum banks, idle until attention starts) so every
                arriving x/w DMA stripe is consumed with 12 matmuls
                (~1.28us) -- close to the ~1.46us/stripe supply rate, so
                the PE tracks the DMA stream with no re-read passes."""
                POOL6 = [(ps1, "ps"), (ps1, "ps"), (sps, "sp"),
                         (sps, "sp"), (avps, "av"), (avps, "av")]
                qoff = [0]

                def quad():
                    off = qoff[0]
                    qoff[0] = (off + 4) % 6
                    return [POOL6[(off + k) % 6][0].tile(
                                [P, NB], f32, tag=POOL6[(off + k) % 6][1],
                                name=f"p0_{off}_{k}")
                            for k in range(4)]
                for wts, dst in ((WQS, "q"), (WKS, "k")):
                    ps = quad()
                    for a in range(EP):
                        wvw = _wv(wts, a)
                        xvw = _xv(0, a)
                        for ft in range(FT):
                            emit3(ps[ft][:], wvw, xvw,
                                  (ft * P, (ft + 1) * P),
                                  first=(a == 0), last=(a == EP - 1))
                    for ft in range(FT):
                        if dst == "q":
                            _drain_q(0, ft, ps[ft])
                        else:
                            _drain_k(0, ft, ps[ft])
                ps = quad()
                for a in range(EP):
                    wvw = _wv(WVS, a)
                    xvw = _xv(0, a)
                    for stl in range(ST):
                        emit3v(ps[stl][:], xvw, wvw, stl,
                               first=(a == 0), last=(a == EP - 1))
                for stl in range(ST):
                    _vp_write(stl, ps[stl])

            def _vp_write(st, ps):
                vview = Vp[st][:].rearrange("p (h c) -> p h c", c=D + 1)
                nc.vector.tensor_copy(
                    vview[:, :, D:D + 1],
                    onesf[:].rearrange("p (h c) -> p h c", c=1))
                nc.vector.scalar_tensor_tensor(
                    vview[:, :, 0:D], ps[:], INV,
                    bvt[:].rearrange("p (h d) -> p h d", d=D),
                    op0=ALU.mult, op1=ALU.add)

            def proj_gen(sb):
                """QKV projection of s-block sb>=1 (all inputs resident).
                Yields between PE chunks; single open psum at a time so the
                shared ps1 ring stays safe under filler interleaving."""
                for ft in range(FT):
                    ps = ps1.tile([P, NB], f32, tag="ps", name=f"psq{ft}_{sb}")
                    for a in range(EP):
                        emit3(ps[:], _wv(WQS, a), _xv(sb, a),
                              (ft * P, (ft + 1) * P),
                              first=(a == 0), last=(a == EP - 1))
                        if a == 1:
                            yield
                    _drain_q(sb, ft, ps)
                    yield
                for ft in range(FT):
                    ps = ps1.tile([P, NB], f32, tag="ps", name=f"psk{ft}_{sb}")
                    for a in range(EP):
                        emit3(ps[:], _wv(WKS, a), _xv(sb, a),
                              (ft * P, (ft + 1) * P),
                              first=(a == 0), last=(a == EP - 1))
                        if a == 1:
                            yield
                    _drain_k(sb, ft, ps)
                    yield
                for stl in range(ST):
                    ps = ps1.tile([P, NB], f32, tag="ps",
                                  name=f"psv{stl}_{sb}")
                    for a in range(EP):
                        emit3v(ps[:], _xv(sb, a), _wv(WVS, a), stl,
                               first=(a == 0), last=(a == EP - 1))
                        if a == 1:
                            yield
                    _vp_write(ST * sb + stl, ps)
                    yield

            def attn_gen(qb):
                """Attention for q-block qb. Yields once per kt step.

                The head-pair loop is software-pipelined: the NEXT pair's
                first score/exp tile is emitted before this pair's AV drain
                and epilogue, so the ACT engine never starves at pair
                boundaries (its backlog gates the final divide chain)."""
                nkt = ST * (qb + 1)
                ATS[qb] = [pans.tile([P, 4 * NB], f8, tag=f"at2{j}",
                                     name=f"at2{j}_{qb}") for j in range(2)]

                def tile_step(hp, kt, qb2=qb):
                    QT2 = QTS[qb2]
                    j = kt - ST * qb2
                    c0 = j * P if j >= 0 else 0
                    # both heads of the pair share one 2-bank psum tile
                    # and a single strided exp call
                    sp = sps.tile([P, 2 * NB], f32, tag="sp",
                                  name=f"sp{qb2}_{hp}_{kt}")
                    if hp < 2:
                        # fp8 DoubleRow scores: one matmul per head,
                        # contracting both d-halves at 0.5 cycles/row
                        k8v = K8[:].rearrange("p (t k) -> p t k", k=S)
                        q8v = QT2[0][:].rearrange("p (t s) -> p t s", s=NB)
                        for i in range(2):
                            h = 2 * hp + i
                            hs = slice(h * 32, (h + 1) * 32)
                            nc.tensor.matmul(
                                sp[:, i * NB + c0:(i + 1) * NB],
                                k8v[hs, :, kt * P:(kt + 1) * P],
                                q8v[hs, :, c0:NB],
                                start=True, stop=True, perf_mode=DR,
                                tile_position=(h * 32, 0))
                    else:
                        for i in range(2):
                            nc.tensor.matmul(
                                sp[:, i * NB + c0:(i + 1) * NB],
                                KT[hp][i * D:(i + 1) * D,
                                       kt * P:(kt + 1) * P],
                                QT2[hp][i * D:(i + 1) * D, c0:NB],
                                start=True, stop=True)
                    w = pwe.tile([P, 2 * NB], bf16, tag="w",
                                 name=f"w{qb2}_{hp}_{kt}")
                    spv = sp[:].rearrange("p (h q) -> p h q", h=2)
                    wv_ = w[:].rearrange("p (h q) -> p h q", h=2)
                    nc.scalar.activation(wv_[:, :, c0:NB],
                                         spv[:, :, c0:NB],
                                         AF.Exp,
                                         scale=(SCALE / (SQ * SQ)
                                                if hp < 2 else SCALE))
                    if j >= 0:
                        nc.vector.tensor_mul(
                            wv_[:, :, c0:c0 + P], wv_[:, :, c0:c0 + P],
                            mtri[:]
                            .rearrange("p (a q) -> p a q", a=1)
                            .broadcast_to([P, 2, P]))
                    return (kt, c0, w)

                hoist = list(XSEED)
                del XSEED[:]
                for hp in range(FT):
                    av = [avps.tile([D + 1, NB], f32, tag="av",
                                    name=f"av{qb}_{hp}_{i}")
                          for i in range(2)]

                    def emit_av(ent, last, av=av, hp=hp):
                        k0, pc0, w0 = ent
                        for i in range(2):
                            nc.tensor.matmul(
                                av[i][:, pc0:NB],
                                Vp[k0][:, (2 * hp + i) * (D + 1):
                                                (2 * hp + i + 1) * (D + 1)],
                                w0[:, i * NB + pc0:(i + 1) * NB],
                                start=(k0 == 0), stop=last)

                    pend = list(hoist)
                    ktlo = len(hoist)
                    hoist = []
                    if hp == 0 and XSEED2:
                        hoist = list(XSEED2)
                        del XSEED2[:]
                    for kt in range(ktlo, nkt):
                        pend.append(tile_step(hp, kt))
                        if len(pend) > 2:
                            emit_av(pend.pop(0), last=False)
                        if kt == nkt - 1:
                            if hp + 1 < FT:
                                hoist.append(
                                    tile_step(hp + 1, len(hoist)))
                            elif (qb + 1 < ST
                                  and len(QTS.get(qb + 1, [])) == FT):
                                XSEED.append(tile_step(0, 0, qb + 1))
                        yield
                    while pend:
                        ent = pend.pop(0)
                        emit_av(ent, last=not pend)
                        if 0 < len(hoist) < min(HOIST, nkt) \
                                and hp + 1 < FT:
                            hoist.append(tile_step(hp + 1, len(hoist)))
                        elif (hp + 1 == FT and qb + 1 < ST
                              and len(QTS.get(qb + 1, [])) == FT):
                            nx = min(XC1, ST * (qb + 2))
                            for _x in range(3):
                                if 0 < len(XSEED) < nx:
                                    XSEED.append(
                                        tile_step(0, len(XSEED), qb + 1))
                        yield
                    # epilogue per head: tmp = copy(av) (frees the psum
                    # slot fast, no divide-chain wait), then in-place
                    # tmp = (tmp*8)/Z; at_hi = fp8(tmp), at_lo =
                    # fp8(tmp - at_hi) rounded pair-wide below.
                    tmpb = pepi.tile([P, NB], bf16, tag="tmpb",
                                     name=f"tmpb{qb}_{hp}")
                    # the very last pair's chain is the exposed tail: route
                    # its copies through ACT (idle after the final exp) and
                    # round hi/lo per head so head 0's rounding overlaps
                    # head 1's divide chain.
                    last_pair = (qb == ST - 1 and hp == FT - 1)
                    on_act = qb <= 1 or last_pair
                    j, jj = hp // 2, hp % 2
                    atv = ATS[qb][j][:].rearrange("p (t s) -> p t s", s=NB)
                    for i in range(2):
                        se = pepi.tile([1, NB], f32, tag="se",
                                       name=f"se{qb}_{hp}_{i}")
                        if on_act:
                            nc.scalar.copy(se[:], av[i][D:D + 1, :])
                            nc.scalar.copy(tmpb[i * D:(i + 1) * D, :],
                                           av[i][0:D, :])
                        else:
                            nc.vector.tensor_copy(se[:], av[i][D:D + 1, :])
                            nc.vector.tensor_copy(
                                tmpb[i * D:(i + 1) * D, :], av[i][0:D, :])
                        nc.vector.reciprocal_approx_fast(se[:], se[:])
                        bch = pepi.tile([P, NB], f32, tag="bch",
                                        name=f"bch{qb}_{hp}_{i}")
                        nc.gpsimd.partition_broadcast(
                            bch[0:(i + 1) * D, :], se[:],
                            channels=(i + 1) * D)
                        nc.vector.scalar_tensor_tensor(
                            tmpb[i * D:(i + 1) * D, :],
                            tmpb[i * D:(i + 1) * D, :], SX,
                            bch[i * D:(i + 1) * D, :],
                            op0=ALU.mult, op1=ALU.mult)
                        if last_pair:
                            hs = slice(i * D, (i + 1) * D)
                            nc.vector.tensor_copy(atv[hs, jj], tmpb[hs, :])
                            nc.vector.tensor_tensor(
                                atv[hs, 2 + jj], tmpb[hs, :], atv[hs, jj],
                                op=ALU.subtract)
                        if (hp + 2 >= FT and qb + 1 < ST
                                and len(QTS.get(qb + 1, [])) == FT):
                            nx = min(XC2, ST * (qb + 2))
                            for _x in range(4):
                                if len(XSEED) < nx \
                                        and (XSEED or hp + 1 == FT):
                                    XSEED.append(
                                        tile_step(0, len(XSEED), qb + 1))
                                elif (len(XSEED) >= nx
                                      and len(XSEED2) < min(
                                          XC3, ST * (qb + 2))):
                                    XSEED2.append(
                                        tile_step(1, len(XSEED2), qb + 1))
                        yield
                    if not last_pair:
                        # pair-wide hi/lo rounding (both heads at once)
                        nc.vector.tensor_copy(atv[:, jj], tmpb[:])
                        nc.vector.tensor_tensor(
                            atv[:, 2 + jj], tmpb[:], atv[:, jj],
                            op=ALU.subtract)
                    yield

            def store_pair(qb, et, ob, eng=None):
                # all loads are issued up-front, so SP.SEQ is free during
                # the main loop; SWDGE stores would block Pool.SEQ (and the
                # softmax broadcasts) while waiting for staging data
                (eng or nc.sync).dma_start(
                    outT[(et - 1) * P:(et + 1) * P,
                         qb * NB:(qb + 1) * NB]
                    .rearrange("(a p) s -> p a s", p=P),
                    ob[:].rearrange("p (a s) -> p a s", s=NB))

            def out_mm(po, qb, et, jset=(0, 1), first=True, last=True):
                """comp DoubleRow out-projection matmuls for e-tile et."""
                for jx, j in enumerate(jset):
                    wvw = WOA[j][:].rearrange("p (t e) -> p t e", e=E)
                    atv = ATS[qb][j][:].rearrange("p (t s) -> p t s", s=NB)
                    c = (et * P, (et + 1) * P)
                    nc.tensor.matmul(
                        po, wvw[:, 0:2, c[0]:c[1]], atv[:, 0:2, :],
                        start=(first and jx == 0), stop=False, perf_mode=DR)
                    nc.tensor.matmul(
                        po, wvw[:, 2:4, c[0]:c[1]], atv[:, 0:2, :],
                        start=False, stop=False, perf_mode=DR)
                    nc.tensor.matmul(
                        po, wvw[:, 0:2, c[0]:c[1]], atv[:, 2:4, :],
                        start=False,
                        stop=(last and jx == len(jset) - 1), perf_mode=DR)

            def out_gen(qb, ets=None, act_copy=False):
                """Output projection of q-block qb. Yields per e-tile.
                Stores are batched in pairs of e-tiles. act_copy routes the
                psum drains through ACT (for tail portions emitted after the
                last exp, when ACT is idle but DVE is still busy)."""
                ob = None
                for et in (range(ET) if ets is None else ets):
                    if et % 2 == 0:
                        ob = pout.tile([P, 2 * NB], bf16, tag="ob",
                                       name=f"ob{qb}_{et}")
                    po = ps1.tile([P, NB], f32, tag="ps",
                                  name=f"po{qb}_{et}")
                    out_mm(po[:], qb, et)
                    if act_copy:
                        nc.scalar.mul(
                            ob[:, (et % 2) * NB:(et % 2 + 1) * NB], po[:],
                            INV)
                    else:
                        nc.vector.tensor_scalar(
                            ob[:, (et % 2) * NB:(et % 2 + 1) * NB], po[:],
                            INV, None, op0=ALU.mult)
                    if et % 2 == 1:
                        store_pair(qb, et, ob)
                    yield

            O3 = {}

            def out3_a():
                """Final-block e-tiles 0-5 open with the j=0 (head pairs
                0-1) halves: pure PE work depending only on those pairs.
                Emitted right after the last pair's AV drain so it executes
                during that pair's divide chain. The open groups borrow
                attention's score psum slots."""
                for et in (0, 1, 2, 3, 4, 5):
                    pool, tg = ((ps1, "ps") if et < 2 else
                                (sps, "sp") if et < 4 else (avps, "av"))
                    po = pool.tile([P, NB], f32, tag=tg, name=f"po3a_{et}")
                    O3[et] = po[:]
                    out_mm(po[:], ST - 1, et, jset=(0,), first=True,
                           last=False)

            def out3():
                """Final block: j=1 closers for e-tiles 0-5, full
                accumulations for e-tiles 6-7, stores batched in pairs with
                single-tile tail stores on alternating queues."""
                qb = ST - 1
                out3_a()
                pos = O3
                ob = None
                for et in range(ET):
                    if et < 6:
                        po = pos[et]
                        out_mm(po, qb, et, jset=(1,), first=False,
                               last=True)
                    else:
                        pool, tg = (ps1, "ps") if et == 6 else (sps, "sp")
                        po = pool.tile([P, NB], f32, tag=tg,
                                       name=f"po3b_{et}")[:]
                        out_mm(po, qb, et)
                    if et < 6:
                        if et % 2 == 0:
                            ob = pout.tile([P, 2 * NB], bf16, tag="ob",
                                           name=f"ob{qb}_{et}")
                            nc.scalar.mul(ob[:, 0:NB], po, INV)
                        else:
                            nc.vector.tensor_scalar(
                                ob[:, NB:2 * NB], po, INV, None,
                                op0=ALU.mult)
                            store_pair(qb, et, ob)
                    else:
                        # drain tail: single-tile stores on alternating
                        # queues so the last transfers issue immediately
                        ob = obt[et - 6]
                        if et == 6:
                            nc.scalar.mul(ob[:], po, INV)
                        else:
                            nc.vector.tensor_scalar(
                                ob[:], po, INV, None, op0=ALU.mult)
                        (nc.scalar if et == 6 else nc.sync).dma_start(
                            outT[et * P:(et + 1) * P,
                                 qb * NB:(qb + 1) * NB], ob[:])

            def drain(g):
                for _ in g:
                    pass

            # warmup: burn the PE p-state ramp while the first input
            # stripes are still in flight, so real matmuls start full-rate
            for i in range(4):
                dp = avps.tile([8, NB], f32, tag="av", name=f"dummy{i}")
                nc.tensor.matmul(dp[:], dum[:, 0:8], dum[:],
                                 start=True, stop=True)
            proj0()
            # Filler plan: spread PE-only work over each attention block to
            # absorb the ACT(exp) deficit; OUT(1)/OUT(2) go to attention(3),
            # which has no projection work left to hide exp latency.
            plans = {
                0: ([lambda: proj_gen(1)], 24),
                1: ([lambda: proj_gen(2)], 24),
                2: ([lambda: proj_gen(3)], 24),
                3: ([lambda: out_gen(0), lambda: out_gen(1),
                     lambda: out_gen(2)], 24),
            }
            for qb in range(ST):
                mk, nf = plans[qb]
                fillers = [m() for m in mk]
                na = 4 * (ST * (qb + 1) + 6)
                fac = FACS[qb]
                rate = fac * nf / na
                acc, fi = 0.0, 0
                for _ in attn_gen(qb):
                    acc += rate
                    while acc >= 1.0 and fillers:
                        acc -= 1.0
                        f = fillers[fi % len(fillers)]
                        fi += 1
                        try:
                            next(f)
                        except StopIteration:
                            fillers.remove(f)
                for f in fillers:
                    drain(f)
            out3()
    nc.compile()
    return nc


def _mask_tri():
    import ml_dtypes
    kp = np.arange(P)[:, None]
    qf = np.arange(P)[None, :]
    return (qf >= kp).astype(ml_dtypes.bfloat16)


def _qk_perm():
    """Column permutation for Wq/Wk: features of heads 0-3 reordered to
    (d-half, 32*h + d%32) so projection psums land in the fp8 DoubleRow
    score layout; heads 4-7 unchanged."""
    perm = np.arange(FQ)
    for newcol in range(2 * P):
        ft, p = divmod(newcol, P)
        h, dd = divmod(p, 32)
        perm[newcol] = h * D + ft * 32 + dd
    return perm


def _qk_bias(bvec, perm):
    """bias vector reordered like the W columns, with the fp8-score heads'
    entries pre-scaled by SQ (their drains fold q8 = SQ*(q + bias))."""
    b = np.ascontiguousarray(bvec.reshape(FQ)[perm])
    b[:2 * P] *= SQ
    return b


def _hilo(a):
    """fp8 hi/lo split: a ~ hi + lo, both e4m3."""
    import ml_dtypes
    f8 = ml_dtypes.float8_e4m3fn
    a = np.ascontiguousarray(a, dtype=np.float32)
    hi = a.astype(f8)
    lo = (a - hi.astype(np.float32)).astype(f8)
    return hi, lo


def _slotpack(hi, lo, cols):
    """[rows=2*P, cols] hi/lo planes -> [P, 4, cols] slot quad
    (hi_e0, hi_e1, lo_e0, lo_e1)."""
    out = np.empty((P, 4, cols), dtype=hi.dtype)
    out[:, 0] = hi[0:P]
    out[:, 1] = hi[P:2 * P]
    out[:, 2] = lo[0:P]
    out[:, 3] = lo[P:2 * P]
    return out


def _pack_w(w):
    """[E or FQ, cols] scaled weight -> [npairs, P, 4*cols] stripe images."""
    hi, lo = _hilo(w)
    n = w.shape[0] // (2 * P)
    return np.stack([
        _slotpack(hi[2 * s * P:(2 * s + 2) * P],
                  lo[2 * s * P:(2 * s + 2) * P],
                  w.shape[1]).reshape(P, -1)
        for s in range(n)])


def _pack_x(xT):
    """[E, S] scaled x^T -> (stripe images [EP, P, 4*NB],
    block images [ST-1, P, 16*NB])."""
    hi, lo = _hilo(xT)
    xs = np.stack([
        _slotpack(hi[2 * s * P:(2 * s + 2) * P, 0:NB],
                  lo[2 * s * P:(2 * s + 2) * P, 0:NB], NB).reshape(P, -1)
        for s in range(EP)])
    xa = np.empty((ST - 1, P, 16 * NB), dtype=hi.dtype)
    for sb in range(1, ST):
        c = slice(sb * NB, (sb + 1) * NB)
        blk = np.stack([
            _slotpack(hi[2 * a * P:(2 * a + 2) * P, c],
                      lo[2 * a * P:(2 * a + 2) * P, c], NB)
            for a in range(EP)], axis=1)          # [P, 4, 4, NB]
        xa[sb - 1] = blk.reshape(P, -1)
    return xs, xa


def kernel(x, W_qkv, b_qkv, W_out, b_out):
    from concourse.bass_utils import run_bass_kernel_spmd

    if "nc" not in _cache:
        _cache["nc"] = _build()
    nc = _cache["nc"]

    x = np.asarray(x, dtype=np.float32)
    W_qkv = np.asarray(W_qkv, dtype=np.float32)
    b_qkv = np.asarray(b_qkv, dtype=np.float32)
    W_out = np.asarray(W_out, dtype=np.float32)
    b_out = np.asarray(b_out, dtype=np.float32)

    mtri = _mask_tri()
    perm = _qk_perm()
    in_maps = []
    for c in range(NCORES):
        b, g = c % B, c // B
        hs = slice(g * HC, (g + 1) * HC)
        Wl = W_qkv[:, :, hs, :]                       # [E, 3, HC, D]
        xs_im, xa_im = _pack_x(x[b].T * SX)
        in_maps.append({
            "xsd": xs_im,
            "xad": xa_im,
            "wqd": _pack_w(Wl[:, 0].reshape(E, FQ)[:, perm] * SW),
            "wkd": _pack_w(Wl[:, 1].reshape(E, FQ)[:, perm] * SW),
            "wvd": _pack_w(Wl[:, 2].reshape(E, FQ) * SW),
            "wod": _pack_w(W_out[hs].reshape(FQ, E) * SW),
            "msk": mtri,
            "bq": _qk_bias(b_qkv[0, hs], perm),
            "bk": _qk_bias(b_qkv[1, hs], perm),
            "bvb": np.broadcast_to(b_qkv[2, hs].reshape(1, FQ),
                                   (P, FQ)).copy(),
        })

    try:
        res = run_bass_kernel_spmd(nc, in_maps, core_ids=list(range(NCORES)))
    except Exception:
        # transient device wedges (NRT_EXEC_UNIT_UNRECOVERABLE) clear on retry
        res = run_bass_kernel_spmd(nc, in_maps, core_ids=list(range(NCORES)))
    _cache["last_results"] = res
    out = np.empty((B, S, E), dtype=np.float32)
    for b in range(B):
        out[b] = (res.results[b]["outT"].T.astype(np.float32)
                  + res.results[b + B]["outT"].T.astype(np.float32)
                  + b_out)
    return out
